# revision 19
# baseline (speedup 1.0000x reference)
"""Trainium2 Bass kernel for nn_Ensembler (nms_detection).

Contract: kernel(**inputs) takes the FULL unsharded inputs
(voxel_logits [3,64,128,128,32] f32, query_logits [3,1,64,21] f32,
sem_prob_dense [21,128,128,32] f32) and returns the FULL output
[64,128,128,32] f32.

Strategy: shard the voxel grids over the flattened voxel dimension
N = X*Y*Z across 8 NeuronCores (each core owns a contiguous slice of
N).  The QxQ IoU statistics are computed as per-shard 0/1-mask GEMMs
(fp8 on the tensor engine) reduced with a tiny AllReduce; the
argmax / matching / merge / keep steps are then replicated on every
core, and the merge + keep + occupancy masking are embarrassingly
parallel over the local N slice.

v3: the data-dependent row gathers aux_v[aux_idx] are indirect DMAs
(SWDGE row gather, device-computed indices) instead of one-hot fp32
matmuls on the PE.  The per-core q-layout is [128 part = (qb, q),
H = NS/2 cols] with n = qb*H + j, so each partition's columns are a
contiguous half-row in DRAM and a single indirect DMA with
idx2 = 2*aux_idx + qb and coef H gathers a full [128, W] window.
Scheduling: the l2 mask loads ride the Pool (SWDGE) queue so they
drain AFTER the pass-B gathers; the occupancy block runs in the AR1
shadow; sigmoid outputs are bf16 (value path only) to deepen buffers.

Numerical notes:
 - all mask decisions are computed from logit signs (exact): the
   iteration-2 anchor mask uses (sig(x0)+sig(x1))/2 > 0.5 <=>
   x0 + x1 > 0, avoiding sigmoid-LUT error in the decision path.
 - sigmoid LUT (ScalarE) max abs err ~3.6e-6 and bf16 prob rounding
   (~4e-3) affect output values only, never matching decisions.
"""

import numpy as np

S = 3
Q = 64
X, Y, Z = 128, 128, 32
N = X * Y * Z           # 524288
C_SEM = 21
NCORES = 8
NS = N // NCORES        # 65536 voxels per core
H = NS // 2             # 32768 cols per partition in q-layout
JP = NS // 128          # 512 contiguous voxels per partition (n-layout)
QC = 4                  # q rows per n-layout read chunk
UC = 2048               # unit cols (16 units, 1:1 with l2 chunks)
NU = H // UC            # 16
LB = 4096               # l0q tile cols (8 tiles)

_compiled = None


def _register_custom_dve_ops():
    """Register two fused DVE ops at runtime (halves the DVE op count on
    the blend/mask hot paths).  Purely additive registration in the
    concourse dve_ops tables; rows stay within the 5-bit byte-36 field."""
    import concourse.dve_ops as dve_ops
    from concourse.dve_ops import DveOp
    from concourse.dve_spec import (Spec, Src0, Src1, C0, C1, Zero, lower,
                                    _has_src1)
    from concourse.dve_uop import DveOpSpec

    if "ANT_BLEND2_K" in dve_ops._SUB_OPCODE_FOR_NAME:
        by = {op.name: op for op in dve_ops.OPS}
        return by["ANT_BLEND2_K"], by["ANT_MASKGT_K"]

    def make(name, spec):
        row = dve_ops._CUSTOM_DVE_ROW_BASE + len(dve_ops.OPS)
        assert row < 0x20
        dve_ops._SUB_OPCODE_FOR_NAME[name] = row
        shas = {}
        for ver in ("v3", "v4"):
            try:
                uops = lower(spec, ver=ver)
                shas[ver] = DveOpSpec(name=name, opcode=row, uops=uops,
                                      rd1_en=_has_src1(spec)).sha(ver)
            except Exception:
                pass
        op = DveOp(name, spec, subdim=False, uops_sha=shas)
        dve_ops.OPS.append(op)
        dve_ops.CUSTOM_DVE_SPECS[name] = spec
        return op

    blend2 = make("ANT_BLEND2_K", Spec(
        body=Src0 * C0 + Src1 * C1,
        reference=lambda in0, in1, s0, s1, imm2: (
            in0.astype(np.float32) * s0 + in1 * s1).astype(np.float32),
    ))
    maskgt = make("ANT_MASKGT_K", Spec(
        body=Zero < (Src0 + Src1 * C0),
        reference=lambda in0, in1, s0, s1, imm2: (
            (in0.astype(np.float32) + in1 * s0) > 0).astype(np.float32),
    ))
    return blend2, maskgt


def _build_program(phases=("A", "AR1", "B", "G2", "AR2", "C"), real_cc=True,
                   loop_k=None):
    import dataclasses
    import concourse.bass as bass
    import concourse.bacc as bacc
    import concourse.mybir as mybir
    import concourse.tile as tile

    phases = set(phases)
    dt = mybir.dt

    BLEND2, MASKGT = _register_custom_dve_ops()

    def dram_view(ap, pattern, offset_elems):
        """Raw [step,count] (element units) view of a DRAM tensor AP."""
        return dataclasses.replace(ap, ap=[list(p) for p in pattern],
                                   offset=offset_elems)

    nc = bacc.Bacc("TRN2", target_bir_lowering=False, debug=False,
                   num_devices=NCORES)

    l0 = nc.dram_tensor("l0", [Q, NS], dt.float32, kind="ExternalInput").ap()
    l1 = nc.dram_tensor("l1", [Q, NS], dt.float32, kind="ExternalInput").ap()
    l2 = nc.dram_tensor("l2", [Q, NS], dt.float32, kind="ExternalInput").ap()
    sem = nc.dram_tensor("sem", [C_SEM, NS], dt.float32,
                         kind="ExternalInput").ap()
    revcnt = nc.dram_tensor("revcnt", [Q, Q], dt.float32,
                            kind="ExternalInput").ap()
    iotap = nc.dram_tensor("iotap", [128, 1], dt.float32,
                           kind="ExternalInput").ap()
    selr = nc.dram_tensor("selr", [Q, 128], dt.float32,
                          kind="ExternalInput").ap()
    out = nc.dram_tensor("out", [Q, NS], dt.float32,
                         kind="ExternalOutput").ap()

    import contextlib

    with tile.TileContext(nc) as tc:
        with (tc.For_i(0, loop_k, 1) if loop_k else
              contextlib.nullcontext()):
            _body(nc, tc, phases, real_cc, dram_view,
                  (l0, l1, l2, sem, revcnt, iotap, selr, out),
                  (BLEND2, MASKGT), mybir, bass)
    nc.compile()
    return nc


def _body(nc, tc, phases, real_cc, dram_view, tensors, custom_ops, mybir,
          bass):
    import dataclasses

    dt = mybir.dt
    Alu = mybir.AluOpType
    Act = mybir.ActivationFunctionType
    l0, l1, l2, sem, revcnt, iotap, selr, out = tensors
    BLEND2, MASKGT = custom_ops

    if True:
        with tc.tile_pool(name="dram", bufs=1, space="DRAM") as dramp, \
             tc.tile_pool(name="psum", bufs=1, space="PSUM") as psump, \
             tc.tile_pool(name="stats", bufs=1) as stp:

            # ---- DRAM scratch ----------------------------------------
            m0_dram = dramp.tile([Q + 1, NS], dt.float8e4)
            ma2_dram = dramp.tile([Q + 1, NS], dt.float8e4)
            occ_dram = dramp.tile([1, NS], dt.float8e4)
            cc_in1 = dramp.tile([Q + 1, Q + 1], dt.float32)
            cc_out1 = dramp.tile([Q + 1, Q + 1], dt.float32)
            cc_in2 = dramp.tile([Q + 1, Q + 1], dt.float32)
            cc_out2 = dramp.tile([Q + 1, Q + 1], dt.float32)

            # ---- small persistent stat tiles -------------------------
            revc = stp.tile([Q, Q], dt.float32)
            nc.sync.dma_start(revc[:], revcnt[:])
            iou_a1 = stp.tile([Q, 1], dt.float32)
            iou_a2 = stp.tile([Q, 1], dt.float32)
            iotp = stp.tile([128, 1], dt.float32)
            nc.sync.dma_start(iotp[:], iotap[:])
            qbv = stp.tile([128, 1], dt.float32)   # 0 for p<64, 1 for p>=64
            nc.vector.tensor_scalar(qbv[:], iotp[:], 63.5, None,
                                    op0=Alu.is_gt)
            # q -> both-halves replicate matrix: selrep = [I64 | I64]
            selrep = stp.tile([Q, 128], dt.float32)
            nc.sync.dma_start(selrep[:], selr[:])
            cb_pp = stp.tile([128, 4], dt.float32)   # [cb, m1, 1-cb, idx]
            c3k_pp = stp.tile([128, 4], dt.float32)  # [c3, keep, 1-c3, idx]
            gidx1 = stp.tile([128, 1], dt.int32)     # 2*aux_idx1 + qb
            gidx2 = stp.tile([128, 1], dt.int32)     # 2*aux_idx2 + qb

            g1_ps = psump.tile([Q + 1, Q + 1], dt.float32)
            g2_ps = psump.tile([Q + 1, Q + 1], dt.float32)

            # indirect-gather DRAM views: [2Q, H] row-contiguous
            l1g_view = dram_view(l1, [[H, 2 * Q], [1, H]], 0)
            l2g_view = dram_view(l2, [[H, 2 * Q], [1, H]], 0)

            # big persistent region: holds L0 logits (q-layout), then
            # anchor2 in place.  8 tiles of LB cols each.
            with tc.tile_pool(name="bigp", bufs=1) as bigp:
                l0q_tiles = []
                for b in range(8):
                    lt = bigp.tile([128, LB], dt.float32, name=f"l0q_{b}")
                    l0q_tiles.append(lt)
                    for qb in range(2):
                        eng = nc.sync if (b + qb) % 2 == 0 else nc.scalar
                        eng.dma_start(
                            lt[qb * Q:(qb + 1) * Q, :],
                            dram_view(l0, [[NS, Q], [1, LB]],
                                      qb * H + b * LB))

                def l0q_slice(u, w):
                    # unit u covers q-layout cols [u*w, (u+1)*w)
                    ti, off = divmod(u * w, LB)
                    return l0q_tiles[ti][:, off:off + w]

                # =====================================================
                # PASS A: m0 masks -> DRAM roundtrip (layout switch);
                #         m1 (SBUF n-layout) -> G1
                # =====================================================
                with tc.tile_pool(name="m0p", bufs=1) as pa:
                    ones_c = pa.tile([128, JP], dt.float8e4)
                    nc.vector.memset(ones_c[:], 1.0)
                    nc.scalar.dma_start(
                        dram_view(m0_dram, [[JP, 128], [1, JP]], Q * NS),
                        ones_c[:])
                    # m0 masks from the q-layout L0 tiles -> m0_dram
                    for b in range(8):
                        m0c = pa.tile([128, LB], dt.float8e4, tag="m0c",
                                      bufs=2)
                        nc.vector.tensor_scalar(
                            m0c[:], l0q_tiles[b][:], 0.0, None,
                            op0=Alu.is_gt)
                        for qb in range(2):
                            weng = nc.scalar if (b + qb) % 2 == 0 else nc.sync
                            weng.dma_start(
                                dram_view(m0_dram, [[NS, Q], [1, LB]],
                                          qb * H + b * LB),
                                m0c[qb * Q:(qb + 1) * Q, :])
                    # m1 masks: n-layout direct to SBUF (j-major + ones col)
                    with tc.tile_pool(name="m1p", bufs=1) as pm1:
                        m1_sb = pm1.tile([128, JP, Q + 1], dt.float8e4)
                        nc.vector.memset(m1_sb[:, :, Q], 1.0)
                        for qc in range(Q // QC):
                            lc = pm1.tile([128, QC, JP], dt.float32,
                                          tag="ldchunk", bufs=2)
                            src = dram_view(l1,
                                            [[JP, 128], [NS, QC], [1, JP]],
                                            qc * QC * NS)
                            ldeng = nc.sync if qc % 2 == 0 else nc.scalar
                            ldeng.dma_start(lc[:], src)
                            nc.vector.tensor_scalar(
                                m1_sb[:, :, qc * QC:(qc + 1) * QC],
                                lc[:].rearrange("p q j -> p j q"), 0.0,
                                None, op0=Alu.is_gt)
                        # G1 GEMM: m0 readback (8 j-slices) x m1_sb
                        for r in range(8):
                            m0t = pm1.tile([128, Q + 1, JP // 8],
                                           dt.float8e4, tag="m0t", bufs=4)
                            reng = nc.sync if r % 2 == 0 else nc.scalar
                            reng.dma_start(
                                m0t[:],
                                dram_view(
                                    m0_dram,
                                    [[JP, 128], [NS, Q + 1], [1, JP // 8]],
                                    r * (JP // 8)))
                            for j in range(JP // 8):
                                gj = r * (JP // 8) + j
                                nc.tensor.matmul(
                                    g1_ps[:], lhsT=m0t[:, :, j],
                                    rhs=m1_sb[:, gj, :],
                                    start=(gj == 0), stop=(gj == JP - 1))

                # occupancy block in the AR1 shadow: sem loads fill the
                # DMA idle window while AR1 runs; DVE reduces are ~9us.
                # occ[n] = (max_{c>=1} sem[c,n] > sem[0,n])
                if "C" in phases:
                    with tc.tile_pool(name="occp", bufs=1) as po:
                        sem0 = po.tile([128, JP], dt.float32)
                        nc.sync.dma_start(
                            sem0[:],
                            dram_view(sem, [[JP, 128], [1, JP]], 0))
                        mx = po.tile([128, JP], dt.float32)
                        nc.sync.dma_start(
                            mx[:],
                            dram_view(sem, [[JP, 128], [1, JP]], NS))
                        for g0 in range(2, C_SEM, 5):
                            rows = min(5, C_SEM - g0)
                            semc = po.tile([128, 5, JP], dt.float32,
                                           tag="semc", bufs=2,
                                           name=f"semg{g0}")
                            nc.scalar.dma_start(
                                semc[:, :rows, :],
                                dram_view(sem,
                                          [[JP, 128], [NS, rows], [1, JP]],
                                          g0 * NS))
                            for k in range(rows):
                                nc.vector.tensor_tensor(
                                    mx[:], mx[:], semc[:, k, :],
                                    op=Alu.max)
                        occ_n = po.tile([128, JP], dt.float8e4)
                        nc.vector.tensor_tensor(occ_n[:], mx[:],
                                                sem0[:], op=Alu.is_gt)
                        nc.sync.dma_start(
                            dram_view(occ_dram, [[JP, 128], [1, JP]], 0),
                            occ_n[:])

                # m2 mask tile persists through G2; fill is interleaved
                # into pass B (loads ride the sync/scalar queues at B's
                # pace; masks lag on the Pool queue).
                pm2 = tc.alloc_tile_pool(name="m2p", bufs=1)
                m2_sb = pm2.tile([128, JP, Q + 1], dt.float8e4)
                nc.vector.memset(m2_sb[:, :, Q], 1.0)

                def m2_fill(dma_eng, mask_eng):
                    with tc.tile_pool(name="m2fill", bufs=1) as pmf:
                        for qc in range(Q // QC):
                            lc2 = pmf.tile([128, QC, JP], dt.float32,
                                           tag="ld2chunk", bufs=2)
                            src = dram_view(l2,
                                            [[JP, 128], [NS, QC], [1, JP]],
                                            qc * QC * NS)
                            dma_eng.dma_start(lc2[:], src)
                            mask_eng.tensor_scalar(
                                m2_sb[:, :, qc * QC:(qc + 1) * QC],
                                lc2[:].rearrange("p q j -> p j q"), 0.0,
                                None, op0=Alu.is_gt)

                # ---- shared stats machinery --------------------------
                def stats_round(g_ps, cc_in, cc_out, iou_a):
                    sfx = cc_in.name
                    gs = stp.tile([Q + 1, Q + 1], dt.float32,
                                  name=f"gs_{sfx}")
                    nc.vector.tensor_copy(gs[:], g_ps[:])
                    nc.sync.dma_start(cc_in[:], gs[:])
                    if real_cc:
                        nc.gpsimd.collective_compute(
                            "AllReduce", Alu.add,
                            replica_groups=[list(range(NCORES))],
                            ins=[cc_in.opt()], outs=[cc_out.opt()])
                    else:
                        nc.sync.dma_start(cc_out[:], cc_in[:])
                    gr = stp.tile([Q + 1, Q + 1], dt.float32,
                                  name=f"gr_{sfx}")
                    nc.sync.dma_start(gr[:], cc_out[:])
                    sbb = stp.tile([Q, Q], dt.float32, name=f"sbb_{sfx}")
                    row = cc_out[Q:Q + 1, 0:Q]
                    nc.sync.dma_start(
                        sbb[:], dataclasses.replace(
                            row, ap=[[0, Q]] + [list(p) for p in row.ap[1:]]))
                    inter = gr[0:Q, 0:Q]
                    sa = gr[0:Q, Q:Q + 1]
                    u = stp.tile([Q, Q], dt.float32, name=f"u_{sfx}")
                    nc.vector.tensor_scalar(u[:], inter, sa, None,
                                            op0=Alu.subtract)
                    nc.vector.tensor_tensor(u[:], sbb[:], u[:],
                                            op=Alu.subtract)
                    nc.vector.tensor_scalar(u[:], u[:], 1.0, None,
                                            op0=Alu.max)
                    nc.vector.reciprocal(u[:], u[:])
                    iou = stp.tile([Q, Q], dt.float32, name=f"iou_{sfx}")
                    nc.vector.tensor_tensor(iou[:], inter, u[:], op=Alu.mult)
                    nc.vector.tensor_reduce(iou_a[:], iou[:],
                                            axis=mybir.AxisListType.X,
                                            op=Alu.max)
                    matched = stp.tile([Q, 1], dt.float32, name=f"mt_{sfx}")
                    nc.vector.tensor_scalar(matched[:], iou_a[:], 0.2, None,
                                            op0=Alu.is_gt)
                    eq = stp.tile([Q, Q], dt.float32, name=f"eq_{sfx}")
                    nc.vector.tensor_scalar(eq[:], iou[:], iou_a[:, 0:1],
                                            None, op0=Alu.is_equal)
                    nc.vector.tensor_tensor(eq[:], eq[:], revc[:],
                                            op=Alu.mult)
                    sm = stp.tile([Q, 1], dt.float32, name=f"sm_{sfx}")
                    nc.vector.tensor_reduce(sm[:], eq[:],
                                            axis=mybir.AxisListType.X,
                                            op=Alu.max)
                    nc.vector.tensor_scalar(sm[:], sm[:], -1.0, float(Q),
                                            op0=Alu.mult, op1=Alu.add)
                    return matched, sm

                def pack_round(matched, col1, sm, w, pp, gidx, tag):
                    """pack [w*m, col1, 1-w*m, sm] and replicate to both
                    q-halves [128, 4] via a PE matmul with selrep; build
                    gidx = 2*sm + qb (int32)."""
                    pk = stp.tile([Q, 4], dt.float32, name=f"pk_{tag}")
                    nc.vector.tensor_scalar(pk[:, 0:1], matched[:], w,
                                            None, op0=Alu.mult)
                    nc.vector.tensor_copy(pk[:, 1:2], col1[:])
                    nc.vector.tensor_scalar(pk[:, 2:3], matched[:], -w,
                                            1.0, op0=Alu.mult, op1=Alu.add)
                    nc.vector.tensor_copy(pk[:, 3:4], sm[:])
                    rep_ps = psump.tile([128, 4], dt.float32,
                                        name=f"reps_{tag}")
                    nc.tensor.matmul(rep_ps[:], lhsT=selrep[:], rhs=pk[:],
                                     start=True, stop=True)
                    nc.vector.tensor_copy(pp[:], rep_ps[:])
                    repi = stp.tile([128, 1], dt.float32, name=f"ri_{tag}")
                    nc.vector.scalar_tensor_tensor(
                        repi[:], pp[:, 3:4], 2.0, qbv[:],
                        op0=Alu.mult, op1=Alu.add)
                    nc.vector.tensor_copy(gidx[:], repi[:])

                if "AR1" in phases:
                    matched1, sm1 = stats_round(g1_ps, cc_in1, cc_out1,
                                                iou_a1)
                    pack_round(matched1, matched1, sm1, 0.5, cb_pp, gidx1,
                               "r1")

                # =====================================================
                # PASS B: indirect gather of l1 rows; anchor2 blend in
                #         place + ma2 mask -> DRAM; l2 masks on Pool
                #         queue (drain after gathers); G2 GEMM
                # =====================================================
                if "B" in phases:
                    with tc.tile_pool(name="blend", bufs=1) as pb:
                        ones_r = pb.tile([128, JP], dt.float8e4)
                        nc.vector.memset(ones_r[:], 1.0)
                        nc.scalar.dma_start(
                            dram_view(ma2_dram, [[JP, 128], [1, JP]],
                                      Q * NS),
                            ones_r[:])

                        for u in range(NU):
                            lgt = pb.tile([128, UC], dt.float32,
                                          tag="lgt", bufs=2)
                            nc.gpsimd.indirect_dma_start(
                                out=lgt[:], out_offset=None,
                                in_=l1g_view,
                                in_offset=bass.IndirectOffsetOnAxis(
                                    ap=gidx1[:, :1], axis=0),
                                element_offset=u * UC)
                            sl = l0q_slice(u, UC)
                            ma2u = pb.tile([128, UC], dt.float8e4,
                                           tag="ma2u", bufs=1)
                            # exact mask (l0 + matched1*l1g) > 0
                            nc.vector._custom_dve(
                                MASKGT, out=ma2u[:], in0=sl, in1=lgt[:],
                                s0=cb_pp[:, 1:2])
                            weng = nc.scalar if u % 2 == 0 else nc.sync
                            weng.dma_start(
                                dram_view(ma2_dram,
                                          [[H, 2], [NS, Q], [1, UC]],
                                          u * UC),
                                ma2u[:])
                            # two l2 n-layout chunks ride along per unit;
                            # their strided mask builds go on Pool, slotting
                            # into its gather-wait gaps.
                            for k in range(2):
                                qc = 2 * u + k
                                lc2 = pb.tile([128, 2, JP], dt.float32,
                                              tag="ld2chunk", bufs=2)
                                ld2e = nc.sync if k == 0 else nc.scalar
                                ld2e.dma_start(
                                    lc2[:],
                                    dram_view(l2,
                                              [[JP, 128], [NS, 2], [1, JP]],
                                              qc * 2 * NS))
                                nc.vector.tensor_scalar(
                                    m2_sb[:, :, qc * 2:(qc + 1) * 2],
                                    lc2[:].rearrange("p q j -> p j q"),
                                    0.0, None, op0=Alu.is_gt)
                            p0c = pb.tile([128, UC], dt.bfloat16,
                                          tag="p0c", bufs=2)
                            nc.scalar.activation(p0c[:], sl, Act.Sigmoid)
                            p1g = pb.tile([128, UC], dt.bfloat16,
                                          tag="p1g", bufs=2)
                            nc.scalar.activation(p1g[:], lgt[:],
                                                 Act.Sigmoid)
                            # anchor2 = (1-cb)*p0 + cb*p1g, in place
                            nc.vector._custom_dve(
                                BLEND2, out=sl, in0=p0c[:], in1=p1g[:],
                                s0=cb_pp[:, 2:3], s1=cb_pp[:, 0:1])
                    if "G2" in phases:
                        with tc.tile_pool(name="g2", bufs=1) as pg:
                            ma2t = pg.tile([128, Q + 1, JP], dt.float8e4)
                            for g in range(8):
                                ps = slice(g * 16, (g + 1) * 16)
                                eng = nc.sync if g % 2 == 0 else nc.scalar
                                eng.dma_start(
                                    ma2t[ps, :, :],
                                    dram_view(
                                        ma2_dram,
                                        [[JP, 16], [NS, Q + 1], [1, JP]],
                                        g * 16 * JP))
                            for j in range(JP):
                                nc.tensor.matmul(
                                    g2_ps[:], lhsT=ma2t[:, :, j],
                                    rhs=m2_sb[:, j, :],
                                    start=(j == 0), stop=(j == JP - 1))
                    pm2.release()

                    if "AR2" in phases:
                        matched2, sm2q = stats_round(g2_ps, cc_in2,
                                                     cc_out2, iou_a2)
                        # keep = mean(iou1, iou2) > 0.2 goes in col 1
                        t64 = stp.tile([Q, 1], dt.float32)
                        nc.vector.tensor_tensor(t64[:], iou_a1[:],
                                                iou_a2[:], op=Alu.add)
                        keep = stp.tile([Q, 1], dt.float32)
                        nc.vector.tensor_scalar(keep[:], t64[:], 0.5,
                                                0.2, op0=Alu.mult,
                                                op1=Alu.is_gt)
                        pack_round(matched2, keep, sm2q, 1.0 / 3.0,
                                   c3k_pp, gidx2, "r2")

                    # =================================================
                    # PASS C: indirect gather of l2 rows; final merge +
                    #         keep + occupancy -> out
                    # =================================================
                    if "C" in phases:
                        with tc.tile_pool(name="passc", bufs=1) as pc:
                            for u in range(NU):
                                lgt2 = pc.tile([128, UC], dt.float32,
                                               tag="lgt2", bufs=4)
                                nc.gpsimd.indirect_dma_start(
                                    out=lgt2[:], out_offset=None,
                                    in_=l2g_view,
                                    in_offset=bass.IndirectOffsetOnAxis(
                                        ap=gidx2[:, :1], axis=0),
                                    element_offset=u * UC)
                                occu = pc.tile([128, UC], dt.float8e4,
                                               tag="occu", bufs=2)
                                oeng = nc.sync if u % 2 == 0 else nc.scalar
                                oeng.dma_start(
                                    occu[:],
                                    dram_view(occ_dram,
                                              [[H, 2], [0, Q], [1, UC]],
                                              u * UC))
                                a2s = l0q_slice(u, UC)
                                # sigmoid in place on the gather tile
                                nc.scalar.activation(lgt2[:], lgt2[:],
                                                     Act.Sigmoid)
                                sm2 = pc.tile([128, UC], dt.float32,
                                              tag="sm2", bufs=2)
                                nc.vector._custom_dve(
                                    BLEND2, out=sm2[:], in0=a2s,
                                    in1=lgt2[:], s0=c3k_pp[:, 2:3],
                                    s1=c3k_pp[:, 0:1])
                                oc = pc.tile([128, UC], dt.float32,
                                             tag="oc", bufs=2)
                                nc.vector.scalar_tensor_tensor(
                                    oc[:], sm2[:], c3k_pp[:, 1:2],
                                    occu[:],
                                    op0=Alu.mult, op1=Alu.mult)
                                weng = nc.sync if u % 2 == 0 else nc.scalar
                                weng.dma_start(
                                    dram_view(out,
                                              [[H, 2], [NS, Q], [1, UC]],
                                              u * UC),
                                    oc[:])

                if "B" not in phases:
                    m2_fill(nc.scalar, nc.vector)
                    pm2.release()
            if "C" not in phases:
                nc.sync.dma_start(
                    dram_view(out, [[NS, Q], [1, Q]], 0), revc[:])


def _get_program():
    global _compiled
    if _compiled is None:
        _compiled = _build_program()
    return _compiled


def _make_in_maps(voxel_logits, sem_prob_dense):
    vl = np.ascontiguousarray(
        np.asarray(voxel_logits, dtype=np.float32).reshape(S, Q, N))
    sp = np.ascontiguousarray(
        np.asarray(sem_prob_dense, dtype=np.float32).reshape(C_SEM, N))
    revcnt = np.tile((Q - np.arange(Q, dtype=np.float32))[None, :], (Q, 1))
    iotap = np.arange(128, dtype=np.float32)[:, None]
    selrep = np.concatenate([np.eye(Q, dtype=np.float32)] * 2, axis=1)
    in_maps = []
    for c in range(NCORES):
        sl = slice(c * NS, (c + 1) * NS)
        in_maps.append({
            "l0": np.ascontiguousarray(vl[0, :, sl]),
            "l1": np.ascontiguousarray(vl[1, :, sl]),
            "l2": np.ascontiguousarray(vl[2, :, sl]),
            "sem": np.ascontiguousarray(sp[:, sl]),
            "revcnt": revcnt,
            "iotap": iotap,
            "selr": selrep,
        })
    return in_maps


def profile_run(inputs):
    """Run once with NTFF tracing; returns exec_time_ns or None."""
    from concourse.bass_utils import run_bass_kernel_spmd

    nc = _get_program()
    in_maps = _make_in_maps(inputs["voxel_logits"], inputs["sem_prob_dense"])
    res = run_bass_kernel_spmd(nc, in_maps, list(range(NCORES)), trace=True)
    return res.exec_time_ns


def kernel(voxel_logits, query_logits, sem_prob_dense):
    from concourse.bass_utils import run_bass_kernel_spmd

    nc = _get_program()
    in_maps = _make_in_maps(voxel_logits, sem_prob_dense)
    res = run_bass_kernel_spmd(nc, in_maps, list(range(NCORES)))
    full = np.concatenate([res.results[c]["out"] for c in range(NCORES)],
                          axis=1)
    return full.reshape(Q, X, Y, Z).astype(np.float32)


# revision 20
# speedup vs baseline: 1.6498x; 1.6498x over previous
"""Trainium2 Bass kernel for nn_Ensembler (nms_detection).

Contract: kernel(**inputs) takes the FULL unsharded inputs
(voxel_logits [3,64,128,128,32] f32, query_logits [3,1,64,21] f32,
sem_prob_dense [21,128,128,32] f32) and returns the FULL output
[64,128,128,32] f32.

Strategy: shard the voxel grids over the flattened voxel dimension
N = X*Y*Z across 8 NeuronCores (each core owns a contiguous slice of
N).  The QxQ IoU statistics are computed as per-shard 0/1-mask GEMMs
(fp8 on the tensor engine) reduced with a tiny AllReduce; the
argmax / matching / merge / keep steps are then replicated on every
core, and the merge + keep + occupancy masking are embarrassingly
parallel over the local N slice.

v3: the data-dependent row gathers aux_v[aux_idx] are indirect DMAs
(SWDGE row gather, device-computed indices) instead of one-hot fp32
matmuls on the PE.  The per-core q-layout is [128 part = (qb, q),
H = NS/2 cols] with n = qb*H + j, so each partition's columns are a
contiguous half-row in DRAM and a single indirect DMA with
idx2 = 2*aux_idx + qb and coef H gathers a full [128, W] window.
Scheduling: the l2 mask loads ride the Pool (SWDGE) queue so they
drain AFTER the pass-B gathers; the occupancy block runs in the AR1
shadow; sigmoid outputs are bf16 (value path only) to deepen buffers.

Numerical notes:
 - all mask decisions are computed from logit signs (exact): the
   iteration-2 anchor mask uses (sig(x0)+sig(x1))/2 > 0.5 <=>
   x0 + x1 > 0, avoiding sigmoid-LUT error in the decision path.
 - sigmoid LUT (ScalarE) max abs err ~3.6e-6 and bf16 prob rounding
   (~4e-3) affect output values only, never matching decisions.
"""

import numpy as np

S = 3
Q = 64
X, Y, Z = 128, 128, 32
N = X * Y * Z           # 524288
C_SEM = 21
NCORES = 8
NS = N // NCORES        # 65536 voxels per core
H = NS // 2             # 32768 cols per partition in q-layout
JP = NS // 128          # 512 contiguous voxels per partition (n-layout)
QC = 4                  # q rows per n-layout read chunk
UC = 2048               # unit cols (16 units, 1:1 with l2 chunks)
NU = H // UC            # 16
LB = 4096               # l0q tile cols (8 tiles)

_compiled = None


def _register_custom_dve_ops():
    """Register two fused DVE ops at runtime (halves the DVE op count on
    the blend/mask hot paths).  Purely additive registration in the
    concourse dve_ops tables; rows stay within the 5-bit byte-36 field."""
    import concourse.dve_ops as dve_ops
    from concourse.dve_ops import DveOp
    from concourse.dve_spec import (Spec, Src0, Src1, C0, C1, Zero, lower,
                                    _has_src1)
    from concourse.dve_uop import DveOpSpec

    if "ANT_BLEND2_K" in dve_ops._SUB_OPCODE_FOR_NAME:
        by = {op.name: op for op in dve_ops.OPS}
        return by["ANT_BLEND2_K"], by["ANT_MASKGT_K"]

    def make(name, spec):
        row = dve_ops._CUSTOM_DVE_ROW_BASE + len(dve_ops.OPS)
        assert row < 0x20
        dve_ops._SUB_OPCODE_FOR_NAME[name] = row
        shas = {}
        for ver in ("v3", "v4"):
            try:
                uops = lower(spec, ver=ver)
                shas[ver] = DveOpSpec(name=name, opcode=row, uops=uops,
                                      rd1_en=_has_src1(spec)).sha(ver)
            except Exception:
                pass
        op = DveOp(name, spec, subdim=False, uops_sha=shas)
        dve_ops.OPS.append(op)
        dve_ops.CUSTOM_DVE_SPECS[name] = spec
        return op

    blend2 = make("ANT_BLEND2_K", Spec(
        body=Src0 * C0 + Src1 * C1,
        reference=lambda in0, in1, s0, s1, imm2: (
            in0.astype(np.float32) * s0 + in1 * s1).astype(np.float32),
    ))
    maskgt = make("ANT_MASKGT_K", Spec(
        body=Zero < (Src0 + Src1 * C0),
        reference=lambda in0, in1, s0, s1, imm2: (
            (in0.astype(np.float32) + in1 * s0) > 0).astype(np.float32),
    ))
    return blend2, maskgt


def _build_program(phases=("A", "AR1", "B", "G2", "AR2", "C"), real_cc=True,
                   loop_k=None):
    import dataclasses
    import concourse.bass as bass
    import concourse.bacc as bacc
    import concourse.mybir as mybir
    import concourse.tile as tile

    phases = set(phases)
    dt = mybir.dt

    BLEND2, MASKGT = _register_custom_dve_ops()

    def dram_view(ap, pattern, offset_elems):
        """Raw [step,count] (element units) view of a DRAM tensor AP."""
        return dataclasses.replace(ap, ap=[list(p) for p in pattern],
                                   offset=offset_elems)

    nc = bacc.Bacc("TRN2", target_bir_lowering=False, debug=False,
                   num_devices=NCORES)

    l0 = nc.dram_tensor("l0", [Q, NS], dt.float32, kind="ExternalInput").ap()
    l1 = nc.dram_tensor("l1", [Q, NS], dt.float32, kind="ExternalInput").ap()
    l2 = nc.dram_tensor("l2", [Q, NS], dt.float32, kind="ExternalInput").ap()
    sem = nc.dram_tensor("sem", [C_SEM, NS], dt.float32,
                         kind="ExternalInput").ap()
    revcnt = nc.dram_tensor("revcnt", [Q, Q], dt.float32,
                            kind="ExternalInput").ap()
    iotap = nc.dram_tensor("iotap", [128, 1], dt.float32,
                           kind="ExternalInput").ap()
    selr = nc.dram_tensor("selr", [Q, 128], dt.float32,
                          kind="ExternalInput").ap()
    out = nc.dram_tensor("out", [Q, NS], dt.float32,
                         kind="ExternalOutput").ap()

    import contextlib

    with tile.TileContext(nc) as tc:
        with (tc.For_i(0, loop_k, 1) if loop_k else
              contextlib.nullcontext()):
            _body(nc, tc, phases, real_cc, dram_view,
                  (l0, l1, l2, sem, revcnt, iotap, selr, out),
                  (BLEND2, MASKGT), mybir, bass)
    nc.compile()
    return nc


def _body(nc, tc, phases, real_cc, dram_view, tensors, custom_ops, mybir,
          bass):
    import dataclasses

    dt = mybir.dt
    Alu = mybir.AluOpType
    Act = mybir.ActivationFunctionType
    l0, l1, l2, sem, revcnt, iotap, selr, out = tensors
    BLEND2, MASKGT = custom_ops

    if True:
        with tc.tile_pool(name="dram", bufs=1, space="DRAM") as dramp, \
             tc.tile_pool(name="psum", bufs=1, space="PSUM") as psump, \
             tc.tile_pool(name="stats", bufs=1) as stp:

            # ---- DRAM scratch ----------------------------------------
            m0_dram = dramp.tile([Q + 1, NS], dt.float8e4)
            ma2_dram = dramp.tile([Q + 1, NS], dt.float8e4)
            occ_dram = dramp.tile([1, NS], dt.float8e4)
            cc_in1 = dramp.tile([Q + 1, Q + 1], dt.float32)
            cc_out1 = dramp.tile([Q + 1, Q + 1], dt.float32)
            cc_in2 = dramp.tile([Q + 1, Q + 1], dt.float32)
            cc_out2 = dramp.tile([Q + 1, Q + 1], dt.float32)

            # ---- small persistent stat tiles -------------------------
            revc = stp.tile([Q, Q], dt.float32)
            nc.sync.dma_start(revc[:], revcnt[:])
            iou_a1 = stp.tile([Q, 1], dt.float32)
            iou_a2 = stp.tile([Q, 1], dt.float32)
            iotp = stp.tile([128, 1], dt.float32)
            nc.sync.dma_start(iotp[:], iotap[:])
            qbv = stp.tile([128, 1], dt.float32)   # 0 for p<64, 1 for p>=64
            nc.vector.tensor_scalar(qbv[:], iotp[:], 63.5, None,
                                    op0=Alu.is_gt)
            # q -> both-halves replicate matrix: selrep = [I64 | I64]
            selrep = stp.tile([Q, 128], dt.float32)
            nc.sync.dma_start(selrep[:], selr[:])
            cb_pp = stp.tile([128, 4], dt.float32)   # [cb, m1, 1-cb, idx]
            c3k_pp = stp.tile([128, 4], dt.float32)  # [c3, keep, 1-c3, idx]
            gidx1 = stp.tile([128, 1], dt.int32)     # 2*aux_idx1 + qb
            gidx2 = stp.tile([128, 1], dt.int32)     # 2*aux_idx2 + qb

            g1_ps = psump.tile([Q + 1, Q + 1], dt.float32)
            g2_ps = psump.tile([Q + 1, Q + 1], dt.float32)

            # indirect-gather DRAM views: [2Q, H] row-contiguous
            l1g_view = dram_view(l1, [[H, 2 * Q], [1, H]], 0)
            l2g_view = dram_view(l2, [[H, 2 * Q], [1, H]], 0)

            # big persistent region: holds L0 logits (q-layout), then
            # anchor2 in place.  8 tiles of LB cols each.
            with tc.tile_pool(name="bigp", bufs=1) as bigp:
                l0q_tiles = []
                for b in range(8):
                    lt = bigp.tile([128, LB], dt.float32, name=f"l0q_{b}")
                    l0q_tiles.append(lt)
                    for qb in range(2):
                        eng = nc.sync if (b + qb) % 2 == 0 else nc.scalar
                        eng.dma_start(
                            lt[qb * Q:(qb + 1) * Q, :],
                            dram_view(l0, [[NS, Q], [1, LB]],
                                      qb * H + b * LB))

                def l0q_slice(u, w):
                    # unit u covers q-layout cols [u*w, (u+1)*w)
                    ti, off = divmod(u * w, LB)
                    return l0q_tiles[ti][:, off:off + w]

                # =====================================================
                # PASS A: m0 masks -> DRAM roundtrip (layout switch);
                #         m1 (SBUF n-layout) -> G1
                # =====================================================
                with tc.tile_pool(name="m0p", bufs=1) as pa:
                    ones_c = pa.tile([128, JP], dt.float8e4)
                    nc.vector.memset(ones_c[:], 1.0)
                    nc.scalar.dma_start(
                        dram_view(m0_dram, [[JP, 128], [1, JP]], Q * NS),
                        ones_c[:])
                    # m0 masks from the q-layout L0 tiles -> m0_dram
                    for b in range(8):
                        m0c = pa.tile([128, LB], dt.float8e4, tag="m0c",
                                      bufs=2)
                        nc.vector.tensor_scalar(
                            m0c[:], l0q_tiles[b][:], 0.0, None,
                            op0=Alu.is_gt)
                        for qb in range(2):
                            weng = nc.scalar if (b + qb) % 2 == 0 else nc.sync
                            weng.dma_start(
                                dram_view(m0_dram, [[NS, Q], [1, LB]],
                                          qb * H + b * LB),
                                m0c[qb * Q:(qb + 1) * Q, :])
                    # m1 masks: n-layout direct to SBUF (j-major + ones col)
                    with tc.tile_pool(name="m1p", bufs=1) as pm1:
                        m1_sb = pm1.tile([128, JP, Q + 1], dt.float8e4)
                        nc.vector.memset(m1_sb[:, :, Q], 1.0)
                        for qc in range(Q // QC):
                            lc = pm1.tile([128, QC, JP], dt.float32,
                                          tag="ldchunk", bufs=2)
                            src = dram_view(l1,
                                            [[JP, 128], [NS, QC], [1, JP]],
                                            qc * QC * NS)
                            ldeng = nc.sync if qc % 2 == 0 else nc.scalar
                            ldeng.dma_start(lc[:], src)
                            nc.vector.tensor_scalar(
                                m1_sb[:, :, qc * QC:(qc + 1) * QC],
                                lc[:].rearrange("p q j -> p j q"), 0.0,
                                None, op0=Alu.is_gt)
                        # G1 GEMM: m0 readback (j-halves) x m1_sb
                        for h in range(2):
                            m0t = pm1.tile([128, Q + 1, JP // 2],
                                           dt.float8e4, tag="m0t", bufs=1)
                            nc.sync.dma_start(
                                m0t[:],
                                dram_view(
                                    m0_dram,
                                    [[JP, 128], [NS, Q + 1], [1, JP // 2]],
                                    h * (JP // 2)))
                            for j in range(JP // 2):
                                gj = h * (JP // 2) + j
                                nc.tensor.matmul(
                                    g1_ps[:], lhsT=m0t[:, :, j],
                                    rhs=m1_sb[:, gj, :],
                                    start=(gj == 0), stop=(gj == JP - 1))

                # occupancy block in the AR1 shadow: sem loads fill the
                # DMA idle window while AR1 runs; DVE reduces are ~9us.
                # occ[n] = (max_{c>=1} sem[c,n] > sem[0,n])
                if "C" in phases:
                    with tc.tile_pool(name="occp", bufs=1) as po:
                        sem0 = po.tile([128, JP], dt.float32)
                        nc.sync.dma_start(
                            sem0[:],
                            dram_view(sem, [[JP, 128], [1, JP]], 0))
                        mx = po.tile([128, JP], dt.float32)
                        nc.sync.dma_start(
                            mx[:],
                            dram_view(sem, [[JP, 128], [1, JP]], NS))
                        for g0 in range(2, C_SEM, 5):
                            rows = min(5, C_SEM - g0)
                            semc = po.tile([128, 5, JP], dt.float32,
                                           tag="semc", bufs=2,
                                           name=f"semg{g0}")
                            nc.scalar.dma_start(
                                semc[:, :rows, :],
                                dram_view(sem,
                                          [[JP, 128], [NS, rows], [1, JP]],
                                          g0 * NS))
                            for k in range(rows):
                                nc.vector.tensor_tensor(
                                    mx[:], mx[:], semc[:, k, :],
                                    op=Alu.max)
                        occ_n = po.tile([128, JP], dt.float8e4)
                        nc.vector.tensor_tensor(occ_n[:], mx[:],
                                                sem0[:], op=Alu.is_gt)
                        nc.sync.dma_start(
                            dram_view(occ_dram, [[JP, 128], [1, JP]], 0),
                            occ_n[:])

                # m2 mask tile persists through G2; fill is interleaved
                # into pass B (loads ride the sync/scalar queues at B's
                # pace; masks lag on the Pool queue).
                pm2 = tc.alloc_tile_pool(name="m2p", bufs=1)
                m2_sb = pm2.tile([128, JP, Q + 1], dt.float8e4)
                nc.vector.memset(m2_sb[:, :, Q], 1.0)

                def m2_fill_v2():
                    with tc.tile_pool(name="m2fill", bufs=1) as pmf:
                        for qc in range(Q // QC):
                            lc2 = pmf.tile([128, QC, JP], dt.float32,
                                           tag="ld2chunk", bufs=2)
                            src = dram_view(l2,
                                            [[JP, 128], [NS, QC], [1, JP]],
                                            qc * QC * NS)
                            ldeng = nc.scalar if qc % 2 == 0 else nc.sync
                            ldeng.dma_start(lc2[:], src)
                            nc.vector.tensor_scalar(
                                m2_sb[:, :, qc * QC:(qc + 1) * QC],
                                lc2[:].rearrange("p q j -> p j q"), 0.0,
                                None, op0=Alu.is_gt)

                m2_fill_v2()

                def m2_fill(dma_eng, mask_eng):
                    with tc.tile_pool(name="m2fill", bufs=1) as pmf:
                        for qc in range(Q // QC):
                            lc2 = pmf.tile([128, QC, JP], dt.float32,
                                           tag="ld2chunk", bufs=2)
                            src = dram_view(l2,
                                            [[JP, 128], [NS, QC], [1, JP]],
                                            qc * QC * NS)
                            dma_eng.dma_start(lc2[:], src)
                            mask_eng.tensor_scalar(
                                m2_sb[:, :, qc * QC:(qc + 1) * QC],
                                lc2[:].rearrange("p q j -> p j q"), 0.0,
                                None, op0=Alu.is_gt)

                # ---- shared stats machinery --------------------------
                def stats_round(g_ps, cc_in, cc_out, iou_a):
                    sfx = cc_in.name
                    gs = stp.tile([Q + 1, Q + 1], dt.float32,
                                  name=f"gs_{sfx}")
                    nc.vector.tensor_copy(gs[:], g_ps[:])
                    nc.sync.dma_start(cc_in[:], gs[:])
                    if real_cc:
                        nc.gpsimd.collective_compute(
                            "AllReduce", Alu.add,
                            replica_groups=[list(range(NCORES))],
                            ins=[cc_in.opt()], outs=[cc_out.opt()])
                    else:
                        nc.sync.dma_start(cc_out[:], cc_in[:])
                    gr = stp.tile([Q + 1, Q + 1], dt.float32,
                                  name=f"gr_{sfx}")
                    nc.sync.dma_start(gr[:], cc_out[:])
                    sbb = stp.tile([Q, Q], dt.float32, name=f"sbb_{sfx}")
                    row = cc_out[Q:Q + 1, 0:Q]
                    nc.sync.dma_start(
                        sbb[:], dataclasses.replace(
                            row, ap=[[0, Q]] + [list(p) for p in row.ap[1:]]))
                    inter = gr[0:Q, 0:Q]
                    sa = gr[0:Q, Q:Q + 1]
                    u = stp.tile([Q, Q], dt.float32, name=f"u_{sfx}")
                    nc.vector.tensor_scalar(u[:], inter, sa, None,
                                            op0=Alu.subtract)
                    nc.vector.tensor_tensor(u[:], sbb[:], u[:],
                                            op=Alu.subtract)
                    nc.vector.tensor_scalar(u[:], u[:], 1.0, None,
                                            op0=Alu.max)
                    nc.vector.reciprocal(u[:], u[:])
                    iou = stp.tile([Q, Q], dt.float32, name=f"iou_{sfx}")
                    nc.vector.tensor_tensor(iou[:], inter, u[:], op=Alu.mult)
                    nc.vector.tensor_reduce(iou_a[:], iou[:],
                                            axis=mybir.AxisListType.X,
                                            op=Alu.max)
                    matched = stp.tile([Q, 1], dt.float32, name=f"mt_{sfx}")
                    nc.vector.tensor_scalar(matched[:], iou_a[:], 0.2, None,
                                            op0=Alu.is_gt)
                    eq = stp.tile([Q, Q], dt.float32, name=f"eq_{sfx}")
                    nc.vector.tensor_scalar(eq[:], iou[:], iou_a[:, 0:1],
                                            None, op0=Alu.is_equal)
                    nc.vector.tensor_tensor(eq[:], eq[:], revc[:],
                                            op=Alu.mult)
                    sm = stp.tile([Q, 1], dt.float32, name=f"sm_{sfx}")
                    nc.vector.tensor_reduce(sm[:], eq[:],
                                            axis=mybir.AxisListType.X,
                                            op=Alu.max)
                    nc.vector.tensor_scalar(sm[:], sm[:], -1.0, float(Q),
                                            op0=Alu.mult, op1=Alu.add)
                    return matched, sm

                def pack_round(matched, col1, sm, w, pp, gidx, tag):
                    """pack [w*m, col1, 1-w*m, sm] and replicate to both
                    q-halves [128, 4] via a PE matmul with selrep; build
                    gidx = 2*sm + qb (int32)."""
                    pk = stp.tile([Q, 4], dt.float32, name=f"pk_{tag}")
                    nc.vector.tensor_scalar(pk[:, 0:1], matched[:], w,
                                            None, op0=Alu.mult)
                    nc.vector.tensor_copy(pk[:, 1:2], col1[:])
                    nc.vector.tensor_scalar(pk[:, 2:3], matched[:], -w,
                                            1.0, op0=Alu.mult, op1=Alu.add)
                    nc.vector.tensor_copy(pk[:, 3:4], sm[:])
                    rep_ps = psump.tile([128, 4], dt.float32,
                                        name=f"reps_{tag}")
                    nc.tensor.matmul(rep_ps[:], lhsT=selrep[:], rhs=pk[:],
                                     start=True, stop=True)
                    nc.vector.tensor_copy(pp[:], rep_ps[:])
                    repi = stp.tile([128, 1], dt.float32, name=f"ri_{tag}")
                    nc.vector.scalar_tensor_tensor(
                        repi[:], pp[:, 3:4], 2.0, qbv[:],
                        op0=Alu.mult, op1=Alu.add)
                    nc.vector.tensor_copy(gidx[:], repi[:])

                if "AR1" in phases:
                    matched1, sm1 = stats_round(g1_ps, cc_in1, cc_out1,
                                                iou_a1)
                    pack_round(matched1, matched1, sm1, 0.5, cb_pp, gidx1,
                               "r1")

                # =====================================================
                # PASS B: indirect gather of l1 rows; anchor2 blend in
                #         place + ma2 mask -> DRAM; l2 masks on Pool
                #         queue (drain after gathers); G2 GEMM
                # =====================================================
                if "B" in phases:
                    with tc.tile_pool(name="blend", bufs=1) as pb:
                        ones_r = pb.tile([128, JP], dt.float8e4)
                        nc.vector.memset(ones_r[:], 1.0)
                        nc.scalar.dma_start(
                            dram_view(ma2_dram, [[JP, 128], [1, JP]],
                                      Q * NS),
                            ones_r[:])

                        for u in range(NU):
                            lgt = pb.tile([128, UC], dt.float32,
                                          tag="lgt", bufs=2)
                            nc.gpsimd.indirect_dma_start(
                                out=lgt[:], out_offset=None,
                                in_=l1g_view,
                                in_offset=bass.IndirectOffsetOnAxis(
                                    ap=gidx1[:, :1], axis=0),
                                element_offset=u * UC)
                            sl = l0q_slice(u, UC)
                            ma2u = pb.tile([128, UC], dt.float8e4,
                                           tag="ma2u", bufs=1)
                            # exact mask (l0 + matched1*l1g) > 0
                            nc.vector._custom_dve(
                                MASKGT, out=ma2u[:], in0=sl, in1=lgt[:],
                                s0=cb_pp[:, 1:2])
                            weng = nc.scalar if u % 2 == 0 else nc.sync
                            weng.dma_start(
                                dram_view(ma2_dram,
                                          [[H, 2], [NS, Q], [1, UC]],
                                          u * UC),
                                ma2u[:])
                            p0c = pb.tile([128, UC], dt.bfloat16,
                                          tag="p0c", bufs=2)
                            nc.scalar.activation(p0c[:], sl, Act.Sigmoid)
                            p1g = pb.tile([128, UC], dt.bfloat16,
                                          tag="p1g", bufs=2)
                            nc.scalar.activation(p1g[:], lgt[:],
                                                 Act.Sigmoid)
                            # anchor2 = (1-cb)*p0 + cb*p1g, in place
                            nc.vector._custom_dve(
                                BLEND2, out=sl, in0=p0c[:], in1=p1g[:],
                                s0=cb_pp[:, 2:3], s1=cb_pp[:, 0:1])
                    if "G2" in phases:
                        with tc.tile_pool(name="g2", bufs=1) as pg:
                            ma2t = pg.tile([128, Q + 1, JP], dt.float8e4)
                            for g in range(8):
                                ps = slice(g * 16, (g + 1) * 16)
                                eng = nc.sync if g % 2 == 0 else nc.scalar
                                eng.dma_start(
                                    ma2t[ps, :, :],
                                    dram_view(
                                        ma2_dram,
                                        [[JP, 16], [NS, Q + 1], [1, JP]],
                                        g * 16 * JP))
                            for j in range(JP):
                                nc.tensor.matmul(
                                    g2_ps[:], lhsT=ma2t[:, :, j],
                                    rhs=m2_sb[:, j, :],
                                    start=(j == 0), stop=(j == JP - 1))
                    pm2.release()

                    if "AR2" in phases:
                        matched2, sm2q = stats_round(g2_ps, cc_in2,
                                                     cc_out2, iou_a2)
                        # keep = mean(iou1, iou2) > 0.2 goes in col 1
                        t64 = stp.tile([Q, 1], dt.float32)
                        nc.vector.tensor_tensor(t64[:], iou_a1[:],
                                                iou_a2[:], op=Alu.add)
                        keep = stp.tile([Q, 1], dt.float32)
                        nc.vector.tensor_scalar(keep[:], t64[:], 0.5,
                                                0.2, op0=Alu.mult,
                                                op1=Alu.is_gt)
                        pack_round(matched2, keep, sm2q, 1.0 / 3.0,
                                   c3k_pp, gidx2, "r2")

                    # =================================================
                    # PASS C: indirect gather of l2 rows; final merge +
                    #         keep + occupancy -> out
                    # =================================================
                    if "C" in phases:
                        with tc.tile_pool(name="passc", bufs=1) as pc:
                            for u in range(NU):
                                lgt2 = pc.tile([128, UC], dt.float32,
                                               tag="lgt2", bufs=4)
                                nc.gpsimd.indirect_dma_start(
                                    out=lgt2[:], out_offset=None,
                                    in_=l2g_view,
                                    in_offset=bass.IndirectOffsetOnAxis(
                                        ap=gidx2[:, :1], axis=0),
                                    element_offset=u * UC)
                                occu = pc.tile([128, UC], dt.float8e4,
                                               tag="occu", bufs=2)
                                oeng = nc.sync if u % 2 == 0 else nc.scalar
                                oeng.dma_start(
                                    occu[:],
                                    dram_view(occ_dram,
                                              [[H, 2], [0, Q], [1, UC]],
                                              u * UC))
                                a2s = l0q_slice(u, UC)
                                # sigmoid in place on the gather tile
                                nc.scalar.activation(lgt2[:], lgt2[:],
                                                     Act.Sigmoid)
                                sm2 = pc.tile([128, UC], dt.float32,
                                              tag="sm2", bufs=2)
                                nc.vector._custom_dve(
                                    BLEND2, out=sm2[:], in0=a2s,
                                    in1=lgt2[:], s0=c3k_pp[:, 2:3],
                                    s1=c3k_pp[:, 0:1])
                                oc = pc.tile([128, UC], dt.float32,
                                             tag="oc", bufs=2)
                                nc.vector.scalar_tensor_tensor(
                                    oc[:], sm2[:], c3k_pp[:, 1:2],
                                    occu[:],
                                    op0=Alu.mult, op1=Alu.mult)
                                weng = nc.sync if u % 2 == 0 else nc.scalar
                                weng.dma_start(
                                    dram_view(out,
                                              [[H, 2], [NS, Q], [1, UC]],
                                              u * UC),
                                    oc[:])

                if "B" not in phases:
                    m2_fill(nc.scalar, nc.vector)
                    pm2.release()
            if "C" not in phases:
                nc.sync.dma_start(
                    dram_view(out, [[NS, Q], [1, Q]], 0), revc[:])


def _get_program():
    global _compiled
    if _compiled is None:
        _compiled = _build_program()
    return _compiled


def _make_in_maps(voxel_logits, sem_prob_dense):
    vl = np.ascontiguousarray(
        np.asarray(voxel_logits, dtype=np.float32).reshape(S, Q, N))
    sp = np.ascontiguousarray(
        np.asarray(sem_prob_dense, dtype=np.float32).reshape(C_SEM, N))
    revcnt = np.tile((Q - np.arange(Q, dtype=np.float32))[None, :], (Q, 1))
    iotap = np.arange(128, dtype=np.float32)[:, None]
    selrep = np.concatenate([np.eye(Q, dtype=np.float32)] * 2, axis=1)
    in_maps = []
    for c in range(NCORES):
        sl = slice(c * NS, (c + 1) * NS)
        in_maps.append({
            "l0": np.ascontiguousarray(vl[0, :, sl]),
            "l1": np.ascontiguousarray(vl[1, :, sl]),
            "l2": np.ascontiguousarray(vl[2, :, sl]),
            "sem": np.ascontiguousarray(sp[:, sl]),
            "revcnt": revcnt,
            "iotap": iotap,
            "selr": selrep,
        })
    return in_maps


def profile_run(inputs):
    """Run once with NTFF tracing; returns exec_time_ns or None."""
    from concourse.bass_utils import run_bass_kernel_spmd

    nc = _get_program()
    in_maps = _make_in_maps(inputs["voxel_logits"], inputs["sem_prob_dense"])
    res = run_bass_kernel_spmd(nc, in_maps, list(range(NCORES)), trace=True)
    return res.exec_time_ns


def kernel(voxel_logits, query_logits, sem_prob_dense):
    from concourse.bass_utils import run_bass_kernel_spmd

    nc = _get_program()
    in_maps = _make_in_maps(voxel_logits, sem_prob_dense)
    res = run_bass_kernel_spmd(nc, in_maps, list(range(NCORES)))
    full = np.concatenate([res.results[c]["out"] for c in range(NCORES)],
                          axis=1)
    return full.reshape(Q, X, Y, Z).astype(np.float32)


# revision 21
# speedup vs baseline: 2.2760x; 1.3796x over previous
"""Trainium2 Bass kernel for nn_Ensembler (nms_detection).

Contract: kernel(**inputs) takes the FULL unsharded inputs
(voxel_logits [3,64,128,128,32] f32, query_logits [3,1,64,21] f32,
sem_prob_dense [21,128,128,32] f32) and returns the FULL output
[64,128,128,32] f32.

Strategy: shard the voxel grids over the flattened voxel dimension
N = X*Y*Z across 8 NeuronCores (each core owns a contiguous slice of
N).  The QxQ IoU statistics are computed as per-shard 0/1-mask GEMMs
(fp8 on the tensor engine) reduced with a tiny AllReduce; the
argmax / matching / merge / keep steps are then replicated on every
core, and the merge + keep + occupancy masking are embarrassingly
parallel over the local N slice.

v3: the data-dependent row gathers aux_v[aux_idx] are indirect DMAs
(SWDGE row gather, device-computed indices) instead of one-hot fp32
matmuls on the PE.  The per-core q-layout is [128 part = (qb, q),
H = NS/2 cols] with n = qb*H + j, so each partition's columns are a
contiguous half-row in DRAM and a single indirect DMA with
idx2 = 2*aux_idx + qb and coef H gathers a full [128, W] window.
Scheduling: the l2 mask loads ride the Pool (SWDGE) queue so they
drain AFTER the pass-B gathers; the occupancy block runs in the AR1
shadow; sigmoid outputs are bf16 (value path only) to deepen buffers.

Numerical notes:
 - all mask decisions are computed from logit signs (exact): the
   iteration-2 anchor mask uses (sig(x0)+sig(x1))/2 > 0.5 <=>
   x0 + x1 > 0, avoiding sigmoid-LUT error in the decision path.
 - sigmoid LUT (ScalarE) max abs err ~3.6e-6 and bf16 prob rounding
   (~4e-3) affect output values only, never matching decisions.
"""

import numpy as np

S = 3
Q = 64
X, Y, Z = 128, 128, 32
N = X * Y * Z           # 524288
C_SEM = 21
NCORES = 8
NS = N // NCORES        # 65536 voxels per core
H = NS // 2             # 32768 cols per partition in q-layout
JP = NS // 128          # 512 contiguous voxels per partition (n-layout)
QC = 4                  # q rows per n-layout read chunk
UC = 2048               # unit cols (16 units, 1:1 with l2 chunks)
NU = H // UC            # 16
LB = 4096               # l0q tile cols (8 tiles)

_compiled = None


def _register_custom_dve_ops():
    """Register two fused DVE ops at runtime (halves the DVE op count on
    the blend/mask hot paths).  Purely additive registration in the
    concourse dve_ops tables; rows stay within the 5-bit byte-36 field."""
    import concourse.dve_ops as dve_ops
    from concourse.dve_ops import DveOp
    from concourse.dve_spec import (Spec, Src0, Src1, C0, C1, Zero, lower,
                                    _has_src1)
    from concourse.dve_uop import DveOpSpec

    if "ANT_BLEND2_K" in dve_ops._SUB_OPCODE_FOR_NAME:
        by = {op.name: op for op in dve_ops.OPS}
        return by["ANT_BLEND2_K"], by["ANT_MASKGT_K"]

    def make(name, spec):
        row = dve_ops._CUSTOM_DVE_ROW_BASE + len(dve_ops.OPS)
        assert row < 0x20
        dve_ops._SUB_OPCODE_FOR_NAME[name] = row
        shas = {}
        for ver in ("v3", "v4"):
            try:
                uops = lower(spec, ver=ver)
                shas[ver] = DveOpSpec(name=name, opcode=row, uops=uops,
                                      rd1_en=_has_src1(spec)).sha(ver)
            except Exception:
                pass
        op = DveOp(name, spec, subdim=False, uops_sha=shas)
        dve_ops.OPS.append(op)
        dve_ops.CUSTOM_DVE_SPECS[name] = spec
        return op

    blend2 = make("ANT_BLEND2_K", Spec(
        body=Src0 * C0 + Src1 * C1,
        reference=lambda in0, in1, s0, s1, imm2: (
            in0.astype(np.float32) * s0 + in1 * s1).astype(np.float32),
    ))
    maskgt = make("ANT_MASKGT_K", Spec(
        body=Zero < (Src0 + Src1 * C0),
        reference=lambda in0, in1, s0, s1, imm2: (
            (in0.astype(np.float32) + in1 * s0) > 0).astype(np.float32),
    ))
    return blend2, maskgt


def _build_program(phases=("A", "AR1", "B", "G2", "AR2", "C"), real_cc=True,
                   loop_k=None):
    import dataclasses
    import concourse.bass as bass
    import concourse.bacc as bacc
    import concourse.mybir as mybir
    import concourse.tile as tile

    phases = set(phases)
    dt = mybir.dt

    BLEND2, MASKGT = _register_custom_dve_ops()

    def dram_view(ap, pattern, offset_elems):
        """Raw [step,count] (element units) view of a DRAM tensor AP."""
        return dataclasses.replace(ap, ap=[list(p) for p in pattern],
                                   offset=offset_elems)

    nc = bacc.Bacc("TRN2", target_bir_lowering=False, debug=False,
                   num_devices=NCORES)

    l0 = nc.dram_tensor("l0", [Q, NS], dt.float32, kind="ExternalInput").ap()
    l1 = nc.dram_tensor("l1", [Q, NS], dt.float32, kind="ExternalInput").ap()
    l2 = nc.dram_tensor("l2", [Q, NS], dt.float32, kind="ExternalInput").ap()
    sem = nc.dram_tensor("sem", [C_SEM, NS], dt.float32,
                         kind="ExternalInput").ap()
    revcnt = nc.dram_tensor("revcnt", [Q, Q], dt.float32,
                            kind="ExternalInput").ap()
    iotap = nc.dram_tensor("iotap", [128, 1], dt.float32,
                           kind="ExternalInput").ap()
    selr = nc.dram_tensor("selr", [Q, 128], dt.float32,
                          kind="ExternalInput").ap()
    out = nc.dram_tensor("out", [Q, NS], dt.float32,
                         kind="ExternalOutput").ap()

    import contextlib

    with tile.TileContext(nc) as tc:
        with (tc.For_i(0, loop_k, 1) if loop_k else
              contextlib.nullcontext()):
            _body(nc, tc, phases, real_cc, dram_view,
                  (l0, l1, l2, sem, revcnt, iotap, selr, out),
                  (BLEND2, MASKGT), mybir, bass)
    nc.compile()
    return nc


def _body(nc, tc, phases, real_cc, dram_view, tensors, custom_ops, mybir,
          bass):
    import dataclasses

    dt = mybir.dt
    Alu = mybir.AluOpType
    Act = mybir.ActivationFunctionType
    l0, l1, l2, sem, revcnt, iotap, selr, out = tensors
    BLEND2, MASKGT = custom_ops

    if True:
        with tc.tile_pool(name="dram", bufs=1, space="DRAM") as dramp, \
             tc.tile_pool(name="psum", bufs=1, space="PSUM") as psump, \
             tc.tile_pool(name="stats", bufs=1) as stp:

            # ---- DRAM scratch ----------------------------------------
            m0_dram = dramp.tile([Q + 1, NS], dt.float8e4)
            ma2_dram = dramp.tile([Q + 1, NS], dt.float8e4)
            occ_dram = dramp.tile([1, NS], dt.float8e4)
            cc_in1 = dramp.tile([Q + 1, Q + 1], dt.float32)
            cc_out1 = dramp.tile([Q + 1, Q + 1], dt.float32)
            cc_in2 = dramp.tile([Q + 1, Q + 1], dt.float32)
            cc_out2 = dramp.tile([Q + 1, Q + 1], dt.float32)

            # ---- small persistent stat tiles -------------------------
            revc = stp.tile([Q, Q], dt.float32)
            nc.sync.dma_start(revc[:], revcnt[:])
            iou_a1 = stp.tile([Q, 1], dt.float32)
            iou_a2 = stp.tile([Q, 1], dt.float32)
            iotp = stp.tile([128, 1], dt.float32)
            nc.sync.dma_start(iotp[:], iotap[:])
            qbv = stp.tile([128, 1], dt.float32)   # 0 for p<64, 1 for p>=64
            nc.vector.tensor_scalar(qbv[:], iotp[:], 63.5, None,
                                    op0=Alu.is_gt)
            # q -> both-halves replicate matrix: selrep = [I64 | I64]
            selrep = stp.tile([Q, 128], dt.float32)
            nc.sync.dma_start(selrep[:], selr[:])
            cb_pp = stp.tile([128, 4], dt.float32)   # [cb, m1, 1-cb, idx]
            c3k_pp = stp.tile([128, 4], dt.float32)  # [c3, keep, 1-c3, idx]
            gidx1 = stp.tile([128, 1], dt.int32)     # 2*aux_idx1 + qb
            gidx2 = stp.tile([128, 1], dt.int32)     # 2*aux_idx2 + qb

            g1_ps = psump.tile([Q + 1, Q + 1], dt.float32)
            g2_ps = psump.tile([Q + 1, Q + 1], dt.float32)

            # indirect-gather DRAM views: [2Q, H] row-contiguous
            l1g_view = dram_view(l1, [[H, 2 * Q], [1, H]], 0)
            l2g_view = dram_view(l2, [[H, 2 * Q], [1, H]], 0)

            # big persistent region: holds L0 logits (q-layout), then
            # anchor2 in place.  8 tiles of LB cols each.
            with tc.tile_pool(name="bigp", bufs=1) as bigp:
                l0q_tiles = []
                for b in range(8):
                    lt = bigp.tile([128, LB], dt.float32, name=f"l0q_{b}")
                    l0q_tiles.append(lt)
                    for qb in range(2):
                        eng = nc.sync if (b + qb) % 2 == 0 else nc.scalar
                        eng.dma_start(
                            lt[qb * Q:(qb + 1) * Q, :],
                            dram_view(l0, [[NS, Q], [1, LB]],
                                      qb * H + b * LB))

                def l0q_slice(u, w):
                    # unit u covers q-layout cols [u*w, (u+1)*w)
                    ti, off = divmod(u * w, LB)
                    return l0q_tiles[ti][:, off:off + w]

                # =====================================================
                # PASS A: m0 masks -> DRAM roundtrip (layout switch);
                #         m1 (SBUF n-layout) -> G1
                # =====================================================
                with tc.tile_pool(name="m0p", bufs=1) as pa:
                    ones_c = pa.tile([128, JP], dt.float8e4)
                    nc.vector.memset(ones_c[:], 1.0)
                    nc.scalar.dma_start(
                        dram_view(m0_dram, [[JP, 128], [1, JP]], Q * NS),
                        ones_c[:])
                    # m0 masks from the q-layout L0 tiles -> m0_dram
                    for b in range(8):
                        m0c = pa.tile([128, LB], dt.float8e4, tag="m0c",
                                      bufs=2)
                        nc.vector.tensor_scalar(
                            m0c[:], l0q_tiles[b][:], 0.0, None,
                            op0=Alu.is_gt)
                        for qb in range(2):
                            weng = nc.scalar if (b + qb) % 2 == 0 else nc.sync
                            weng.dma_start(
                                dram_view(m0_dram, [[NS, Q], [1, LB]],
                                          qb * H + b * LB),
                                m0c[qb * Q:(qb + 1) * Q, :])
                    # m1 masks: n-layout direct to SBUF (j-major + ones col)
                    with tc.tile_pool(name="m1p", bufs=1) as pm1:
                        m1_sb = pm1.tile([128, JP, Q + 1], dt.float8e4)
                        nc.vector.memset(m1_sb[:, :, Q], 1.0)
                        for qc in range(Q // QC):
                            lc = pm1.tile([128, QC, JP], dt.float32,
                                          tag="ldchunk", bufs=2)
                            src = dram_view(l1,
                                            [[JP, 128], [NS, QC], [1, JP]],
                                            qc * QC * NS)
                            ldeng = nc.sync if qc % 2 == 0 else nc.scalar
                            ldeng.dma_start(lc[:], src)
                            nc.vector.tensor_scalar(
                                m1_sb[:, :, qc * QC:(qc + 1) * QC],
                                lc[:].rearrange("p q j -> p j q"), 0.0,
                                None, op0=Alu.is_gt)
                        # G1 GEMM: m0 readback (j-halves) x m1_sb
                        for h in range(2):
                            m0t = pm1.tile([128, Q + 1, JP // 2],
                                           dt.float8e4, tag="m0t", bufs=1)
                            nc.sync.dma_start(
                                m0t[:],
                                dram_view(
                                    m0_dram,
                                    [[JP, 128], [NS, Q + 1], [1, JP // 2]],
                                    h * (JP // 2)))
                            for j in range(JP // 2):
                                gj = h * (JP // 2) + j
                                nc.tensor.matmul(
                                    g1_ps[:], lhsT=m0t[:, :, j],
                                    rhs=m1_sb[:, gj, :],
                                    start=(gj == 0), stop=(gj == JP - 1))

                # occupancy block in the AR1 shadow: sem loads fill the
                # DMA idle window while AR1 runs; DVE reduces are ~9us.
                # occ[n] = (max_{c>=1} sem[c,n] > sem[0,n])
                if "C" in phases:
                    with tc.tile_pool(name="occp", bufs=1) as po:
                        sem0 = po.tile([128, JP], dt.float32)
                        nc.sync.dma_start(
                            sem0[:],
                            dram_view(sem, [[JP, 128], [1, JP]], 0))
                        mx = po.tile([128, JP], dt.float32)
                        nc.sync.dma_start(
                            mx[:],
                            dram_view(sem, [[JP, 128], [1, JP]], NS))
                        for g0 in range(2, C_SEM, 5):
                            rows = min(5, C_SEM - g0)
                            semc = po.tile([128, 5, JP], dt.float32,
                                           tag="semc", bufs=2,
                                           name=f"semg{g0}")
                            nc.scalar.dma_start(
                                semc[:, :rows, :],
                                dram_view(sem,
                                          [[JP, 128], [NS, rows], [1, JP]],
                                          g0 * NS))
                            for k in range(rows):
                                nc.vector.tensor_tensor(
                                    mx[:], mx[:], semc[:, k, :],
                                    op=Alu.max)
                        occ_n = po.tile([128, JP], dt.float8e4)
                        nc.vector.tensor_tensor(occ_n[:], mx[:],
                                                sem0[:], op=Alu.is_gt)
                        nc.sync.dma_start(
                            dram_view(occ_dram, [[JP, 128], [1, JP]], 0),
                            occ_n[:])

                # m2 mask tile persists through G2; fill is interleaved
                # into pass B (loads ride the sync/scalar queues at B's
                # pace; masks lag on the Pool queue).
                pm2 = tc.alloc_tile_pool(name="m2p", bufs=1)
                m2_sb = pm2.tile([128, JP, Q + 1], dt.float8e4)
                nc.vector.memset(m2_sb[:, :, Q], 1.0)

                def m2_fill_v2():
                    with tc.tile_pool(name="m2fill", bufs=1) as pmf:
                        for qc in range(Q // QC):
                            lc2 = pmf.tile([128, QC, JP], dt.float32,
                                           tag="ld2chunk", bufs=2)
                            src = dram_view(l2,
                                            [[JP, 128], [NS, QC], [1, JP]],
                                            qc * QC * NS)
                            ldeng = nc.scalar if qc % 2 == 0 else nc.sync
                            ldeng.dma_start(lc2[:], src)
                            nc.vector.tensor_scalar(
                                m2_sb[:, :, qc * QC:(qc + 1) * QC],
                                lc2[:].rearrange("p q j -> p j q"), 0.0,
                                None, op0=Alu.is_gt)

                def m2_fill(dma_eng, mask_eng):
                    with tc.tile_pool(name="m2fill", bufs=1) as pmf:
                        for qc in range(Q // QC):
                            lc2 = pmf.tile([128, QC, JP], dt.float32,
                                           tag="ld2chunk", bufs=2)
                            src = dram_view(l2,
                                            [[JP, 128], [NS, QC], [1, JP]],
                                            qc * QC * NS)
                            dma_eng.dma_start(lc2[:], src)
                            mask_eng.tensor_scalar(
                                m2_sb[:, :, qc * QC:(qc + 1) * QC],
                                lc2[:].rearrange("p q j -> p j q"), 0.0,
                                None, op0=Alu.is_gt)

                # ---- shared stats machinery --------------------------
                def stats_round(g_ps, cc_in, cc_out, iou_a):
                    sfx = cc_in.name
                    gs = stp.tile([Q + 1, Q + 1], dt.float32,
                                  name=f"gs_{sfx}")
                    nc.vector.tensor_copy(gs[:], g_ps[:])
                    nc.sync.dma_start(cc_in[:], gs[:])
                    if real_cc:
                        nc.gpsimd.collective_compute(
                            "AllReduce", Alu.add,
                            replica_groups=[list(range(NCORES))],
                            ins=[cc_in.opt()], outs=[cc_out.opt()])
                    else:
                        nc.sync.dma_start(cc_out[:], cc_in[:])
                    gr = stp.tile([Q + 1, Q + 1], dt.float32,
                                  name=f"gr_{sfx}")
                    nc.sync.dma_start(gr[:], cc_out[:])
                    sbb = stp.tile([Q, Q], dt.float32, name=f"sbb_{sfx}")
                    row = cc_out[Q:Q + 1, 0:Q]
                    nc.sync.dma_start(
                        sbb[:], dataclasses.replace(
                            row, ap=[[0, Q]] + [list(p) for p in row.ap[1:]]))
                    inter = gr[0:Q, 0:Q]
                    sa = gr[0:Q, Q:Q + 1]
                    u = stp.tile([Q, Q], dt.float32, name=f"u_{sfx}")
                    nc.vector.tensor_scalar(u[:], inter, sa, None,
                                            op0=Alu.subtract)
                    nc.vector.tensor_tensor(u[:], sbb[:], u[:],
                                            op=Alu.subtract)
                    nc.vector.tensor_scalar(u[:], u[:], 1.0, None,
                                            op0=Alu.max)
                    nc.vector.reciprocal(u[:], u[:])
                    iou = stp.tile([Q, Q], dt.float32, name=f"iou_{sfx}")
                    nc.vector.tensor_tensor(iou[:], inter, u[:], op=Alu.mult)
                    nc.vector.tensor_reduce(iou_a[:], iou[:],
                                            axis=mybir.AxisListType.X,
                                            op=Alu.max)
                    matched = stp.tile([Q, 1], dt.float32, name=f"mt_{sfx}")
                    nc.vector.tensor_scalar(matched[:], iou_a[:], 0.2, None,
                                            op0=Alu.is_gt)
                    eq = stp.tile([Q, Q], dt.float32, name=f"eq_{sfx}")
                    nc.vector.tensor_scalar(eq[:], iou[:], iou_a[:, 0:1],
                                            None, op0=Alu.is_equal)
                    nc.vector.tensor_tensor(eq[:], eq[:], revc[:],
                                            op=Alu.mult)
                    sm = stp.tile([Q, 1], dt.float32, name=f"sm_{sfx}")
                    nc.vector.tensor_reduce(sm[:], eq[:],
                                            axis=mybir.AxisListType.X,
                                            op=Alu.max)
                    nc.vector.tensor_scalar(sm[:], sm[:], -1.0, float(Q),
                                            op0=Alu.mult, op1=Alu.add)
                    return matched, sm

                def pack_round(matched, col1, sm, w, pp, gidx, tag):
                    """pack [w*m, col1, 1-w*m, sm] and replicate to both
                    q-halves [128, 4] via a PE matmul with selrep; build
                    gidx = 2*sm + qb (int32)."""
                    pk = stp.tile([Q, 4], dt.float32, name=f"pk_{tag}")
                    nc.vector.tensor_scalar(pk[:, 0:1], matched[:], w,
                                            None, op0=Alu.mult)
                    nc.vector.tensor_copy(pk[:, 1:2], col1[:])
                    nc.vector.tensor_scalar(pk[:, 2:3], matched[:], -w,
                                            1.0, op0=Alu.mult, op1=Alu.add)
                    nc.vector.tensor_copy(pk[:, 3:4], sm[:])
                    rep_ps = psump.tile([128, 4], dt.float32,
                                        name=f"reps_{tag}")
                    nc.tensor.matmul(rep_ps[:], lhsT=selrep[:], rhs=pk[:],
                                     start=True, stop=True)
                    nc.vector.tensor_copy(pp[:], rep_ps[:])
                    repi = stp.tile([128, 1], dt.float32, name=f"ri_{tag}")
                    nc.vector.scalar_tensor_tensor(
                        repi[:], pp[:, 3:4], 2.0, qbv[:],
                        op0=Alu.mult, op1=Alu.add)
                    nc.vector.tensor_copy(gidx[:], repi[:])

                if "AR1" in phases:
                    matched1, sm1 = stats_round(g1_ps, cc_in1, cc_out1,
                                                iou_a1)
                    pack_round(matched1, matched1, sm1, 0.5, cb_pp, gidx1,
                               "r1")

                # =====================================================
                # PASS B: indirect gather of l1 rows; anchor2 blend in
                #         place + ma2 mask -> DRAM; l2 masks on Pool
                #         queue (drain after gathers); G2 GEMM
                # =====================================================
                if "B" in phases:
                    with tc.tile_pool(name="blend", bufs=1) as pb:
                        ones_r = pb.tile([128, JP], dt.float8e4)
                        nc.vector.memset(ones_r[:], 1.0)
                        nc.scalar.dma_start(
                            dram_view(ma2_dram, [[JP, 128], [1, JP]],
                                      Q * NS),
                            ones_r[:])

                        for u in range(NU):
                            lgt = pb.tile([128, UC], dt.float32,
                                          tag="lgt", bufs=2)
                            nc.gpsimd.indirect_dma_start(
                                out=lgt[:], out_offset=None,
                                in_=l1g_view,
                                in_offset=bass.IndirectOffsetOnAxis(
                                    ap=gidx1[:, :1], axis=0),
                                element_offset=u * UC)
                            sl = l0q_slice(u, UC)
                            ma2u = pb.tile([128, UC], dt.float8e4,
                                           tag="ma2u", bufs=1)
                            # exact mask (l0 + matched1*l1g) > 0
                            nc.vector._custom_dve(
                                MASKGT, out=ma2u[:], in0=sl, in1=lgt[:],
                                s0=cb_pp[:, 1:2])
                            weng = nc.scalar if u % 2 == 0 else nc.sync
                            weng.dma_start(
                                dram_view(ma2_dram,
                                          [[H, 2], [NS, Q], [1, UC]],
                                          u * UC),
                                ma2u[:])
                            # two l2 n-layout chunks ride along per unit;
                            # masks on DVE between the unit's custom ops
                            for k in range(2):
                                qc = 2 * u + k
                                lc2 = pb.tile([128, 2, JP], dt.float32,
                                              tag="ld2chunk", bufs=2)
                                ld2e = nc.sync if k == 0 else nc.scalar
                                ld2e.dma_start(
                                    lc2[:],
                                    dram_view(l2,
                                              [[JP, 128], [NS, 2], [1, JP]],
                                              qc * 2 * NS))
                                nc.vector.tensor_scalar(
                                    m2_sb[:, :, qc * 2:(qc + 1) * 2],
                                    lc2[:].rearrange("p q j -> p j q"),
                                    0.0, None, op0=Alu.is_gt)
                            p0c = pb.tile([128, UC], dt.bfloat16,
                                          tag="p0c", bufs=2)
                            nc.scalar.activation(p0c[:], sl, Act.Sigmoid)
                            p1g = pb.tile([128, UC], dt.bfloat16,
                                          tag="p1g", bufs=2)
                            nc.scalar.activation(p1g[:], lgt[:],
                                                 Act.Sigmoid)
                            # anchor2 = (1-cb)*p0 + cb*p1g, in place
                            nc.vector._custom_dve(
                                BLEND2, out=sl, in0=p0c[:], in1=p1g[:],
                                s0=cb_pp[:, 2:3], s1=cb_pp[:, 0:1])
                    if "G2" in phases:
                        with tc.tile_pool(name="g2", bufs=1) as pg:
                            ma2t = pg.tile([128, Q + 1, JP], dt.float8e4)
                            for g in range(8):
                                ps = slice(g * 16, (g + 1) * 16)
                                eng = nc.sync if g % 2 == 0 else nc.scalar
                                eng.dma_start(
                                    ma2t[ps, :, :],
                                    dram_view(
                                        ma2_dram,
                                        [[JP, 16], [NS, Q + 1], [1, JP]],
                                        g * 16 * JP))
                            for j in range(JP):
                                nc.tensor.matmul(
                                    g2_ps[:], lhsT=ma2t[:, :, j],
                                    rhs=m2_sb[:, j, :],
                                    start=(j == 0), stop=(j == JP - 1))
                    pm2.release()

                    if "AR2" in phases:
                        matched2, sm2q = stats_round(g2_ps, cc_in2,
                                                     cc_out2, iou_a2)
                        # keep = mean(iou1, iou2) > 0.2 goes in col 1
                        t64 = stp.tile([Q, 1], dt.float32)
                        nc.vector.tensor_tensor(t64[:], iou_a1[:],
                                                iou_a2[:], op=Alu.add)
                        keep = stp.tile([Q, 1], dt.float32)
                        nc.vector.tensor_scalar(keep[:], t64[:], 0.5,
                                                0.2, op0=Alu.mult,
                                                op1=Alu.is_gt)
                        pack_round(matched2, keep, sm2q, 1.0 / 3.0,
                                   c3k_pp, gidx2, "r2")

                    # =================================================
                    # PASS C: indirect gather of l2 rows; final merge +
                    #         keep + occupancy -> out
                    # =================================================
                    if "C" in phases:
                        with tc.tile_pool(name="passc", bufs=1) as pc:
                            for u in range(NU):
                                lgt2 = pc.tile([128, UC], dt.float32,
                                               tag="lgt2", bufs=4)
                                nc.gpsimd.indirect_dma_start(
                                    out=lgt2[:], out_offset=None,
                                    in_=l2g_view,
                                    in_offset=bass.IndirectOffsetOnAxis(
                                        ap=gidx2[:, :1], axis=0),
                                    element_offset=u * UC)
                                occu = pc.tile([128, UC], dt.float8e4,
                                               tag="occu", bufs=2)
                                oeng = nc.sync if u % 2 == 0 else nc.scalar
                                oeng.dma_start(
                                    occu[:],
                                    dram_view(occ_dram,
                                              [[H, 2], [0, Q], [1, UC]],
                                              u * UC))
                                a2s = l0q_slice(u, UC)
                                # sigmoid in place on the gather tile
                                nc.scalar.activation(lgt2[:], lgt2[:],
                                                     Act.Sigmoid)
                                sm2 = pc.tile([128, UC], dt.float32,
                                              tag="sm2", bufs=2)
                                nc.vector._custom_dve(
                                    BLEND2, out=sm2[:], in0=a2s,
                                    in1=lgt2[:], s0=c3k_pp[:, 2:3],
                                    s1=c3k_pp[:, 0:1])
                                oc = pc.tile([128, UC], dt.float32,
                                             tag="oc", bufs=2)
                                nc.vector.scalar_tensor_tensor(
                                    oc[:], sm2[:], c3k_pp[:, 1:2],
                                    occu[:],
                                    op0=Alu.mult, op1=Alu.mult)
                                weng = nc.sync if u % 2 == 0 else nc.scalar
                                weng.dma_start(
                                    dram_view(out,
                                              [[H, 2], [NS, Q], [1, UC]],
                                              u * UC),
                                    oc[:])

                if "B" not in phases:
                    m2_fill(nc.scalar, nc.vector)
                    pm2.release()
            if "C" not in phases:
                nc.sync.dma_start(
                    dram_view(out, [[NS, Q], [1, Q]], 0), revc[:])


def _get_program():
    global _compiled
    if _compiled is None:
        _compiled = _build_program()
    return _compiled


def _make_in_maps(voxel_logits, sem_prob_dense):
    vl = np.ascontiguousarray(
        np.asarray(voxel_logits, dtype=np.float32).reshape(S, Q, N))
    sp = np.ascontiguousarray(
        np.asarray(sem_prob_dense, dtype=np.float32).reshape(C_SEM, N))
    revcnt = np.tile((Q - np.arange(Q, dtype=np.float32))[None, :], (Q, 1))
    iotap = np.arange(128, dtype=np.float32)[:, None]
    selrep = np.concatenate([np.eye(Q, dtype=np.float32)] * 2, axis=1)
    in_maps = []
    for c in range(NCORES):
        sl = slice(c * NS, (c + 1) * NS)
        in_maps.append({
            "l0": np.ascontiguousarray(vl[0, :, sl]),
            "l1": np.ascontiguousarray(vl[1, :, sl]),
            "l2": np.ascontiguousarray(vl[2, :, sl]),
            "sem": np.ascontiguousarray(sp[:, sl]),
            "revcnt": revcnt,
            "iotap": iotap,
            "selr": selrep,
        })
    return in_maps


def profile_run(inputs):
    """Run once with NTFF tracing; returns exec_time_ns or None."""
    from concourse.bass_utils import run_bass_kernel_spmd

    nc = _get_program()
    in_maps = _make_in_maps(inputs["voxel_logits"], inputs["sem_prob_dense"])
    res = run_bass_kernel_spmd(nc, in_maps, list(range(NCORES)), trace=True)
    return res.exec_time_ns


def kernel(voxel_logits, query_logits, sem_prob_dense):
    from concourse.bass_utils import run_bass_kernel_spmd

    nc = _get_program()
    in_maps = _make_in_maps(voxel_logits, sem_prob_dense)
    res = run_bass_kernel_spmd(nc, in_maps, list(range(NCORES)))
    full = np.concatenate([res.results[c]["out"] for c in range(NCORES)],
                          axis=1)
    return full.reshape(Q, X, Y, Z).astype(np.float32)


# revision 27
# speedup vs baseline: 2.6599x; 1.1687x over previous
"""Trainium2 Bass kernel for nn_Ensembler (nms_detection).

Contract: kernel(**inputs) takes the FULL unsharded inputs
(voxel_logits [3,64,128,128,32] f32, query_logits [3,1,64,21] f32,
sem_prob_dense [21,128,128,32] f32) and returns the FULL output
[64,128,128,32] f32.

Strategy: shard the voxel grids over the flattened voxel dimension
N = X*Y*Z across 8 NeuronCores (each core owns a contiguous slice of
N).  The QxQ IoU statistics are computed as per-shard 0/1-mask GEMMs
(fp8 on the tensor engine) reduced with a tiny AllReduce; the
argmax / matching / merge / keep steps are then replicated on every
core, and the merge + keep + occupancy masking are embarrassingly
parallel over the local N slice.

v3: the data-dependent row gathers aux_v[aux_idx] are indirect DMAs
(SWDGE row gather, device-computed indices) instead of one-hot fp32
matmuls on the PE.  The per-core q-layout is [128 part = (qb, q),
H = NS/2 cols] with n = qb*H + j, so each partition's columns are a
contiguous half-row in DRAM and a single indirect DMA with
idx2 = 2*aux_idx + qb and coef H gathers a full [128, W] window.
Scheduling: the l2 mask loads ride the Pool (SWDGE) queue so they
drain AFTER the pass-B gathers; the occupancy block runs in the AR1
shadow; sigmoid outputs are bf16 (value path only) to deepen buffers.

Numerical notes:
 - all mask decisions are computed from logit signs (exact): the
   iteration-2 anchor mask uses (sig(x0)+sig(x1))/2 > 0.5 <=>
   x0 + x1 > 0, avoiding sigmoid-LUT error in the decision path.
 - sigmoid LUT (ScalarE) max abs err ~3.6e-6 and bf16 prob rounding
   (~4e-3) affect output values only, never matching decisions.
"""

import numpy as np

S = 3
Q = 64
X, Y, Z = 128, 128, 32
N = X * Y * Z           # 524288
C_SEM = 21
NCORES = 8
NS = N // NCORES        # 65536 voxels per core
H = NS // 2             # 32768 cols per partition in q-layout
JP = NS // 128          # 512 contiguous voxels per partition (n-layout)
QC = 4                  # q rows per n-layout read chunk
UC = 2048               # unit cols (16 units, 1:1 with l2 chunks)
NU = H // UC            # 16
LB = 4096               # l0q tile cols (8 tiles)

_compiled = None


def _register_custom_dve_ops():
    """Register two fused DVE ops at runtime (halves the DVE op count on
    the blend/mask hot paths).  Purely additive registration in the
    concourse dve_ops tables; rows stay within the 5-bit byte-36 field."""
    import concourse.dve_ops as dve_ops
    from concourse.dve_ops import DveOp
    from concourse.dve_spec import (Spec, Src0, Src1, C0, C1, Zero, lower,
                                    _has_src1)
    from concourse.dve_uop import DveOpSpec

    if "ANT_BLEND2_K" in dve_ops._SUB_OPCODE_FOR_NAME:
        by = {op.name: op for op in dve_ops.OPS}
        return by["ANT_BLEND2_K"], by["ANT_MASKGT_K"]

    def make(name, spec):
        row = dve_ops._CUSTOM_DVE_ROW_BASE + len(dve_ops.OPS)
        assert row < 0x20
        dve_ops._SUB_OPCODE_FOR_NAME[name] = row
        shas = {}
        for ver in ("v3", "v4"):
            try:
                uops = lower(spec, ver=ver)
                shas[ver] = DveOpSpec(name=name, opcode=row, uops=uops,
                                      rd1_en=_has_src1(spec)).sha(ver)
            except Exception:
                pass
        op = DveOp(name, spec, subdim=False, uops_sha=shas)
        dve_ops.OPS.append(op)
        dve_ops.CUSTOM_DVE_SPECS[name] = spec
        return op

    blend2 = make("ANT_BLEND2_K", Spec(
        body=Src0 * C0 + Src1 * C1,
        reference=lambda in0, in1, s0, s1, imm2: (
            in0.astype(np.float32) * s0 + in1 * s1).astype(np.float32),
    ))
    maskgt = make("ANT_MASKGT_K", Spec(
        body=Zero < (Src0 + Src1 * C0),
        reference=lambda in0, in1, s0, s1, imm2: (
            (in0.astype(np.float32) + in1 * s0) > 0).astype(np.float32),
    ))
    return blend2, maskgt


def _build_program(phases=("A", "AR1", "B", "G2", "AR2", "C"), real_cc=True,
                   loop_k=None):
    import dataclasses
    import concourse.bass as bass
    import concourse.bacc as bacc
    import concourse.mybir as mybir
    import concourse.tile as tile

    phases = set(phases)
    dt = mybir.dt

    BLEND2, MASKGT = _register_custom_dve_ops()

    def dram_view(ap, pattern, offset_elems):
        """Raw [step,count] (element units) view of a DRAM tensor AP."""
        return dataclasses.replace(ap, ap=[list(p) for p in pattern],
                                   offset=offset_elems)

    nc = bacc.Bacc("TRN2", target_bir_lowering=False, debug=False,
                   num_devices=NCORES)

    l0 = nc.dram_tensor("l0", [Q, NS], dt.float32, kind="ExternalInput").ap()
    l1 = nc.dram_tensor("l1", [Q, NS], dt.float32, kind="ExternalInput").ap()
    l2 = nc.dram_tensor("l2", [Q, NS], dt.float32, kind="ExternalInput").ap()
    sem = nc.dram_tensor("sem", [C_SEM, NS], dt.float32,
                         kind="ExternalInput").ap()
    revcnt = nc.dram_tensor("revcnt", [Q, Q], dt.float32,
                            kind="ExternalInput").ap()
    iotap = nc.dram_tensor("iotap", [128, 1], dt.float32,
                           kind="ExternalInput").ap()
    selr = nc.dram_tensor("selr", [Q, 128], dt.float32,
                          kind="ExternalInput").ap()
    out = nc.dram_tensor("out", [Q, NS], dt.float32,
                         kind="ExternalOutput").ap()

    import contextlib

    with tile.TileContext(nc) as tc:
        with (tc.For_i(0, loop_k, 1) if loop_k else
              contextlib.nullcontext()):
            _body(nc, tc, phases, real_cc, dram_view,
                  (l0, l1, l2, sem, revcnt, iotap, selr, out),
                  (BLEND2, MASKGT), mybir, bass)
    nc.compile()
    return nc


def _body(nc, tc, phases, real_cc, dram_view, tensors, custom_ops, mybir,
          bass):
    import dataclasses

    dt = mybir.dt
    Alu = mybir.AluOpType
    Act = mybir.ActivationFunctionType
    l0, l1, l2, sem, revcnt, iotap, selr, out = tensors
    BLEND2, MASKGT = custom_ops

    if True:
        with tc.tile_pool(name="dram", bufs=1, space="DRAM") as dramp, \
             tc.tile_pool(name="psum", bufs=1, space="PSUM") as psump, \
             tc.tile_pool(name="stats", bufs=1) as stp:

            # ---- DRAM scratch ----------------------------------------
            ma2_dram = dramp.tile([Q + 1, NS], dt.float8e4)
            occ_dram = dramp.tile([1, NS], dt.float8e4)
            cc_in1 = dramp.tile([Q + 1, Q + 1], dt.float32)
            cc_out1 = dramp.tile([Q + 1, Q + 1], dt.float32)
            cc_in2 = dramp.tile([Q + 1, Q + 1], dt.float32)
            cc_out2 = dramp.tile([Q + 1, Q + 1], dt.float32)

            # ---- small persistent stat tiles -------------------------
            revc = stp.tile([Q, Q], dt.float32)
            nc.sync.dma_start(revc[:], revcnt[:])
            iou_a1 = stp.tile([Q, 1], dt.float32)
            iou_a2 = stp.tile([Q, 1], dt.float32)
            iotp = stp.tile([128, 1], dt.float32)
            nc.sync.dma_start(iotp[:], iotap[:])
            qbv = stp.tile([128, 1], dt.float32)   # 0 for p<64, 1 for p>=64
            nc.vector.tensor_scalar(qbv[:], iotp[:], 63.5, None,
                                    op0=Alu.is_gt)
            # q -> both-halves replicate matrix: selrep = [I64 | I64]
            selrep = stp.tile([Q, 128], dt.float32)
            nc.sync.dma_start(selrep[:], selr[:])
            cb_pp = stp.tile([128, 4], dt.float32)   # [cb, m1, 1-cb, idx]
            c3k_pp = stp.tile([128, 4], dt.float32)  # [c3, keep, 1-c3, idx]
            gidx1 = stp.tile([128, 1], dt.int32)     # 2*aux_idx1 + qb
            gidx2 = stp.tile([128, 1], dt.int32)     # 2*aux_idx2 + qb

            g1_ps = psump.tile([Q + 1, Q + 1], dt.float32)
            g2_ps = psump.tile([Q + 1, Q + 1], dt.float32)

            # indirect-gather DRAM views: [2Q, H] row-contiguous
            l1g_view = dram_view(l1, [[H, 2 * Q], [1, H]], 0)
            l2g_view = dram_view(l2, [[H, 2 * Q], [1, H]], 0)

            # big persistent region: holds L0 logits (q-layout), then
            # anchor2 in place.  8 tiles of LB cols each.
            with tc.tile_pool(name="bigp", bufs=1) as bigp:
                l0q_tiles = []
                for b in range(8):
                    lt = bigp.tile([128, LB], dt.float32, name=f"l0q_{b}")
                    l0q_tiles.append(lt)

                def l0q_slice(u, w):
                    # unit u covers q-layout cols [u*w, (u+1)*w)
                    ti, off = divmod(u * w, LB)
                    return l0q_tiles[ti][:, off:off + w]

                # =====================================================
                # PASS A: l0 and l1 stream in q-layout; BOTH mask sets
                # are PE-transposed slab-by-slab (consistent contiguous
                # n = slab*128 + c map on both G1 operands), masked on
                # the PSUM->SBUF copy.  m0 slabs persist in m0n; m1
                # slabs stream through small stages straight into G1.
                # =====================================================
                with tc.tile_pool(name="m0p", bufs=1) as pa, \
                     tc.tile_pool(name="m1p", bufs=1) as pm1, \
                     tc.tile_pool(name="ps0", bufs=1, space="PSUM") as pps:
                    m0n = pa.tile([128, JP, Q + 1], dt.float8e4)
                    nc.vector.memset(m0n[:, :, Q], 1.0)
                    identf = pa.tile([128, Q], dt.float32)
                    for qb in range(2):
                        nc.sync.dma_start(
                            identf[qb * Q:(qb + 1) * Q, :],
                            dram_view(selr, [[128, Q], [1, Q]], 0))
                    nmm = [0]
                    for b in range(8):
                        lt = l0q_tiles[b]
                        l1q = pm1.tile([128, LB], dt.float32, tag="l1q",
                                       bufs=2)
                        for qb in range(2):
                            eng = nc.sync if (b + qb) % 2 == 0 else nc.scalar
                            eng.dma_start(
                                lt[qb * Q:(qb + 1) * Q, :],
                                dram_view(l0, [[NS, Q], [1, LB]],
                                          qb * H + b * LB))
                            eng2 = nc.scalar if (b + qb) % 2 == 0 else nc.sync
                            eng2.dma_start(
                                l1q[qb * Q:(qb + 1) * Q, :],
                                dram_view(l1, [[NS, Q], [1, LB]],
                                          qb * H + b * LB))
                        for qb in range(2):
                            idq = identf[qb * Q:(qb + 1) * Q, :]
                            for g8 in range(4):
                                tp0 = pps.tile([128, 8, Q], dt.float32,
                                               tag="tp", bufs=4)
                                tp1 = pps.tile([128, 8, Q], dt.float32,
                                               tag="tp", bufs=4)
                                for i in range(8):
                                    s = g8 * 8 + i
                                    cs = slice(s * 128, (s + 1) * 128)
                                    qs = slice(qb * Q, (qb + 1) * Q)
                                    nc.tensor.matmul(
                                        tp0[:, i, :], lhsT=lt[qs, cs],
                                        rhs=idq, is_transpose=True,
                                        start=True, stop=True)
                                    nc.tensor.matmul(
                                        tp1[:, i, :], lhsT=l1q[qs, cs],
                                        rhs=idq, is_transpose=True,
                                        start=True, stop=True)
                                jb = qb * 256 + b * 32 + g8 * 8
                                nc.vector.tensor_scalar(
                                    m0n[:, jb:jb + 8, 0:Q], tp0[:], 0.0,
                                    None, op0=Alu.is_gt)
                                rst = pm1.tile([128, 8, Q + 1],
                                               dt.float8e4, tag="rst",
                                               bufs=3)
                                nc.vector.memset(rst[:, :, Q], 1.0)
                                nc.vector.tensor_scalar(
                                    rst[:, :, 0:Q], tp1[:], 0.0,
                                    None, op0=Alu.is_gt)
                                for i in range(8):
                                    nc.tensor.matmul(
                                        g1_ps[:],
                                        lhsT=m0n[:, jb + i, :],
                                        rhs=rst[:, i, :],
                                        start=(nmm[0] == 0),
                                        stop=(nmm[0] == JP - 1))
                                    nmm[0] += 1

                # occupancy block in the AR1 shadow: sem loads fill the
                # DMA idle window while AR1 runs; DVE reduces are ~9us.
                # occ[n] = (max_{c>=1} sem[c,n] > sem[0,n])
                if "C" in phases:
                    with tc.tile_pool(name="occp", bufs=1) as po:
                        sem0 = po.tile([128, JP], dt.float32)
                        nc.sync.dma_start(
                            sem0[:],
                            dram_view(sem, [[JP, 128], [1, JP]], 0))
                        mx = po.tile([128, JP], dt.float32)
                        nc.sync.dma_start(
                            mx[:],
                            dram_view(sem, [[JP, 128], [1, JP]], NS))
                        for g0 in range(2, C_SEM, 5):
                            rows = min(5, C_SEM - g0)
                            semc = po.tile([128, 5, JP], dt.float32,
                                           tag="semc", bufs=2,
                                           name=f"semg{g0}")
                            nc.scalar.dma_start(
                                semc[:, :rows, :],
                                dram_view(sem,
                                          [[JP, 128], [NS, rows], [1, JP]],
                                          g0 * NS))
                            for k in range(rows):
                                nc.vector.tensor_tensor(
                                    mx[:], mx[:], semc[:, k, :],
                                    op=Alu.max)
                        occ_n = po.tile([128, JP], dt.float8e4)
                        nc.vector.tensor_tensor(occ_n[:], mx[:],
                                                sem0[:], op=Alu.is_gt)
                        nc.sync.dma_start(
                            dram_view(occ_dram, [[JP, 128], [1, JP]], 0),
                            occ_n[:])

                # m2 mask tile persists through G2; fill is interleaved
                # into pass B (loads ride the sync/scalar queues at B's
                # pace; masks lag on the Pool queue).
                pm2 = tc.alloc_tile_pool(name="m2p", bufs=1)
                m2_sb = pm2.tile([128, JP, Q + 1], dt.float8e4)
                nc.vector.memset(m2_sb[:, :, Q], 1.0)

                def m2_fill_v2():
                    with tc.tile_pool(name="m2fill", bufs=1) as pmf:
                        for qc in range(Q // QC):
                            lc2 = pmf.tile([128, QC, JP], dt.float32,
                                           tag="ld2chunk", bufs=2)
                            src = dram_view(l2,
                                            [[JP, 128], [NS, QC], [1, JP]],
                                            qc * QC * NS)
                            ldeng = nc.scalar if qc % 2 == 0 else nc.sync
                            ldeng.dma_start(lc2[:], src)
                            nc.vector.tensor_scalar(
                                m2_sb[:, :, qc * QC:(qc + 1) * QC],
                                lc2[:].rearrange("p q j -> p j q"), 0.0,
                                None, op0=Alu.is_gt)

                def m2_fill(dma_eng, mask_eng):
                    with tc.tile_pool(name="m2fill", bufs=1) as pmf:
                        for qc in range(Q // QC):
                            lc2 = pmf.tile([128, QC, JP], dt.float32,
                                           tag="ld2chunk", bufs=2)
                            src = dram_view(l2,
                                            [[JP, 128], [NS, QC], [1, JP]],
                                            qc * QC * NS)
                            dma_eng.dma_start(lc2[:], src)
                            mask_eng.tensor_scalar(
                                m2_sb[:, :, qc * QC:(qc + 1) * QC],
                                lc2[:].rearrange("p q j -> p j q"), 0.0,
                                None, op0=Alu.is_gt)

                # ---- shared stats machinery --------------------------
                def stats_round(g_ps, cc_in, cc_out, iou_a):
                    sfx = cc_in.name
                    gs = stp.tile([Q + 1, Q + 1], dt.float32,
                                  name=f"gs_{sfx}")
                    nc.vector.tensor_copy(gs[:], g_ps[:])
                    nc.sync.dma_start(cc_in[:], gs[:])
                    if real_cc:
                        nc.gpsimd.collective_compute(
                            "AllReduce", Alu.add,
                            replica_groups=[list(range(NCORES))],
                            ins=[cc_in.opt()], outs=[cc_out.opt()])
                    else:
                        nc.sync.dma_start(cc_out[:], cc_in[:])
                    gr = stp.tile([Q + 1, Q + 1], dt.float32,
                                  name=f"gr_{sfx}")
                    nc.sync.dma_start(gr[:], cc_out[:])
                    sbb = stp.tile([Q, Q], dt.float32, name=f"sbb_{sfx}")
                    row = cc_out[Q:Q + 1, 0:Q]
                    nc.sync.dma_start(
                        sbb[:], dataclasses.replace(
                            row, ap=[[0, Q]] + [list(p) for p in row.ap[1:]]))
                    inter = gr[0:Q, 0:Q]
                    sa = gr[0:Q, Q:Q + 1]
                    u = stp.tile([Q, Q], dt.float32, name=f"u_{sfx}")
                    nc.vector.tensor_scalar(u[:], inter, sa, None,
                                            op0=Alu.subtract)
                    nc.vector.tensor_tensor(u[:], sbb[:], u[:],
                                            op=Alu.subtract)
                    nc.vector.tensor_scalar(u[:], u[:], 1.0, None,
                                            op0=Alu.max)
                    nc.vector.reciprocal(u[:], u[:])
                    iou = stp.tile([Q, Q], dt.float32, name=f"iou_{sfx}")
                    nc.vector.tensor_tensor(iou[:], inter, u[:], op=Alu.mult)
                    nc.vector.tensor_reduce(iou_a[:], iou[:],
                                            axis=mybir.AxisListType.X,
                                            op=Alu.max)
                    matched = stp.tile([Q, 1], dt.float32, name=f"mt_{sfx}")
                    nc.vector.tensor_scalar(matched[:], iou_a[:], 0.2, None,
                                            op0=Alu.is_gt)
                    eq = stp.tile([Q, Q], dt.float32, name=f"eq_{sfx}")
                    nc.vector.tensor_scalar(eq[:], iou[:], iou_a[:, 0:1],
                                            None, op0=Alu.is_equal)
                    nc.vector.tensor_tensor(eq[:], eq[:], revc[:],
                                            op=Alu.mult)
                    sm = stp.tile([Q, 1], dt.float32, name=f"sm_{sfx}")
                    nc.vector.tensor_reduce(sm[:], eq[:],
                                            axis=mybir.AxisListType.X,
                                            op=Alu.max)
                    nc.vector.tensor_scalar(sm[:], sm[:], -1.0, float(Q),
                                            op0=Alu.mult, op1=Alu.add)
                    return matched, sm

                def pack_round(matched, col1, sm, w, pp, gidx, tag):
                    """pack [w*m, col1, 1-w*m, sm] and replicate to both
                    q-halves [128, 4] via a PE matmul with selrep; build
                    gidx = 2*sm + qb (int32)."""
                    pk = stp.tile([Q, 4], dt.float32, name=f"pk_{tag}")
                    nc.vector.tensor_scalar(pk[:, 0:1], matched[:], w,
                                            None, op0=Alu.mult)
                    nc.vector.tensor_copy(pk[:, 1:2], col1[:])
                    nc.vector.tensor_scalar(pk[:, 2:3], matched[:], -w,
                                            1.0, op0=Alu.mult, op1=Alu.add)
                    nc.vector.tensor_copy(pk[:, 3:4], sm[:])
                    rep_ps = psump.tile([128, 4], dt.float32,
                                        name=f"reps_{tag}")
                    nc.tensor.matmul(rep_ps[:], lhsT=selrep[:], rhs=pk[:],
                                     start=True, stop=True)
                    nc.vector.tensor_copy(pp[:], rep_ps[:])
                    repi = stp.tile([128, 1], dt.float32, name=f"ri_{tag}")
                    nc.vector.scalar_tensor_tensor(
                        repi[:], pp[:, 3:4], 2.0, qbv[:],
                        op0=Alu.mult, op1=Alu.add)
                    nc.vector.tensor_copy(gidx[:], repi[:])

                if "AR1" in phases:
                    matched1, sm1 = stats_round(g1_ps, cc_in1, cc_out1,
                                                iou_a1)
                    pack_round(matched1, matched1, sm1, 0.5, cb_pp, gidx1,
                               "r1")

                # =====================================================
                # PASS B: indirect gather of l1 rows; anchor2 blend in
                #         place + ma2 mask -> DRAM; l2 masks on Pool
                #         queue (drain after gathers); G2 GEMM
                # =====================================================
                if "B" in phases:
                    with tc.tile_pool(name="blend", bufs=1) as pb:
                        ones_r = pb.tile([128, JP], dt.float8e4)
                        nc.vector.memset(ones_r[:], 1.0)
                        nc.scalar.dma_start(
                            dram_view(ma2_dram, [[JP, 128], [1, JP]],
                                      Q * NS),
                            ones_r[:])

                        for u in range(NU):
                            lgt = pb.tile([128, UC], dt.float32,
                                          tag="lgt", bufs=2)
                            nc.gpsimd.indirect_dma_start(
                                out=lgt[:], out_offset=None,
                                in_=l1g_view,
                                in_offset=bass.IndirectOffsetOnAxis(
                                    ap=gidx1[:, :1], axis=0),
                                element_offset=u * UC)
                            sl = l0q_slice(u, UC)
                            ma2u = pb.tile([128, UC], dt.float8e4,
                                           tag="ma2u", bufs=1)
                            # exact mask (l0 + matched1*l1g) > 0
                            nc.vector._custom_dve(
                                MASKGT, out=ma2u[:], in0=sl, in1=lgt[:],
                                s0=cb_pp[:, 1:2])
                            weng = nc.scalar if u % 2 == 0 else nc.sync
                            weng.dma_start(
                                dram_view(ma2_dram,
                                          [[H, 2], [NS, Q], [1, UC]],
                                          u * UC),
                                ma2u[:])
                            # two l2 n-layout chunks ride along per unit;
                            # masks on DVE between the unit's custom ops
                            for k in range(2):
                                qc = 2 * u + k
                                lc2 = pb.tile([128, 2, JP], dt.float32,
                                              tag="ld2chunk", bufs=2)
                                ld2e = nc.sync if k == 0 else nc.scalar
                                ld2e.dma_start(
                                    lc2[:],
                                    dram_view(l2,
                                              [[JP, 128], [NS, 2], [1, JP]],
                                              qc * 2 * NS))
                                nc.vector.tensor_scalar(
                                    m2_sb[:, :, qc * 2:(qc + 1) * 2],
                                    lc2[:].rearrange("p q j -> p j q"),
                                    0.0, None, op0=Alu.is_gt)
                            p0c = pb.tile([128, UC], dt.bfloat16,
                                          tag="p0c", bufs=2)
                            nc.scalar.activation(p0c[:], sl, Act.Sigmoid)
                            p1g = pb.tile([128, UC], dt.bfloat16,
                                          tag="p1g", bufs=2)
                            nc.scalar.activation(p1g[:], lgt[:],
                                                 Act.Sigmoid)
                            # anchor2 = (1-cb)*p0 + cb*p1g, in place
                            nc.vector._custom_dve(
                                BLEND2, out=sl, in0=p0c[:], in1=p1g[:],
                                s0=cb_pp[:, 2:3], s1=cb_pp[:, 0:1])
                    if "G2" in phases:
                        with tc.tile_pool(name="g2", bufs=1) as pg:
                            ma2t = pg.tile([128, Q + 1, JP], dt.float8e4)
                            for g in range(8):
                                ps = slice(g * 16, (g + 1) * 16)
                                eng = nc.sync if g % 2 == 0 else nc.scalar
                                eng.dma_start(
                                    ma2t[ps, :, :],
                                    dram_view(
                                        ma2_dram,
                                        [[JP, 16], [NS, Q + 1], [1, JP]],
                                        g * 16 * JP))
                            for j in range(JP):
                                nc.tensor.matmul(
                                    g2_ps[:], lhsT=ma2t[:, :, j],
                                    rhs=m2_sb[:, j, :],
                                    start=(j == 0), stop=(j == JP - 1))
                    pm2.release()

                    if "AR2" in phases:
                        matched2, sm2q = stats_round(g2_ps, cc_in2,
                                                     cc_out2, iou_a2)
                        # keep = mean(iou1, iou2) > 0.2 goes in col 1
                        t64 = stp.tile([Q, 1], dt.float32)
                        nc.vector.tensor_tensor(t64[:], iou_a1[:],
                                                iou_a2[:], op=Alu.add)
                        keep = stp.tile([Q, 1], dt.float32)
                        nc.vector.tensor_scalar(keep[:], t64[:], 0.5,
                                                0.2, op0=Alu.mult,
                                                op1=Alu.is_gt)
                        pack_round(matched2, keep, sm2q, 1.0 / 3.0,
                                   c3k_pp, gidx2, "r2")

                    # =================================================
                    # PASS C: indirect gather of l2 rows; final merge +
                    #         keep + occupancy -> out
                    # =================================================
                    if "C" in phases:
                        with tc.tile_pool(name="passc", bufs=1) as pc:
                            for u in range(NU):
                                lgt2 = pc.tile([128, UC], dt.float32,
                                               tag="lgt2", bufs=4)
                                nc.gpsimd.indirect_dma_start(
                                    out=lgt2[:], out_offset=None,
                                    in_=l2g_view,
                                    in_offset=bass.IndirectOffsetOnAxis(
                                        ap=gidx2[:, :1], axis=0),
                                    element_offset=u * UC)
                                occu = pc.tile([128, UC], dt.float8e4,
                                               tag="occu", bufs=2)
                                oeng = nc.sync if u % 2 == 0 else nc.scalar
                                oeng.dma_start(
                                    occu[:],
                                    dram_view(occ_dram,
                                              [[H, 2], [0, Q], [1, UC]],
                                              u * UC))
                                a2s = l0q_slice(u, UC)
                                # sigmoid in place on the gather tile
                                nc.scalar.activation(lgt2[:], lgt2[:],
                                                     Act.Sigmoid)
                                sm2 = pc.tile([128, UC], dt.float32,
                                              tag="sm2", bufs=2)
                                nc.vector._custom_dve(
                                    BLEND2, out=sm2[:], in0=a2s,
                                    in1=lgt2[:], s0=c3k_pp[:, 2:3],
                                    s1=c3k_pp[:, 0:1])
                                oc = pc.tile([128, UC], dt.float32,
                                             tag="oc", bufs=2)
                                nc.vector.scalar_tensor_tensor(
                                    oc[:], sm2[:], c3k_pp[:, 1:2],
                                    occu[:],
                                    op0=Alu.mult, op1=Alu.mult)
                                weng = nc.sync if u % 2 == 0 else nc.scalar
                                weng.dma_start(
                                    dram_view(out,
                                              [[H, 2], [NS, Q], [1, UC]],
                                              u * UC),
                                    oc[:])

                if "B" not in phases:
                    m2_fill(nc.scalar, nc.vector)
                    pm2.release()
            if "C" not in phases:
                nc.sync.dma_start(
                    dram_view(out, [[NS, Q], [1, Q]], 0), revc[:])


def _get_program():
    global _compiled
    if _compiled is None:
        _compiled = _build_program()
    return _compiled


def _make_in_maps(voxel_logits, sem_prob_dense):
    vl = np.ascontiguousarray(
        np.asarray(voxel_logits, dtype=np.float32).reshape(S, Q, N))
    sp = np.ascontiguousarray(
        np.asarray(sem_prob_dense, dtype=np.float32).reshape(C_SEM, N))
    revcnt = np.tile((Q - np.arange(Q, dtype=np.float32))[None, :], (Q, 1))
    iotap = np.arange(128, dtype=np.float32)[:, None]
    selrep = np.concatenate([np.eye(Q, dtype=np.float32)] * 2, axis=1)
    in_maps = []
    for c in range(NCORES):
        sl = slice(c * NS, (c + 1) * NS)
        in_maps.append({
            "l0": np.ascontiguousarray(vl[0, :, sl]),
            "l1": np.ascontiguousarray(vl[1, :, sl]),
            "l2": np.ascontiguousarray(vl[2, :, sl]),
            "sem": np.ascontiguousarray(sp[:, sl]),
            "revcnt": revcnt,
            "iotap": iotap,
            "selr": selrep,
        })
    return in_maps


def profile_run(inputs):
    """Run once with NTFF tracing; returns exec_time_ns or None."""
    from concourse.bass_utils import run_bass_kernel_spmd

    nc = _get_program()
    in_maps = _make_in_maps(inputs["voxel_logits"], inputs["sem_prob_dense"])
    res = run_bass_kernel_spmd(nc, in_maps, list(range(NCORES)), trace=True)
    return res.exec_time_ns


def kernel(voxel_logits, query_logits, sem_prob_dense):
    from concourse.bass_utils import run_bass_kernel_spmd

    nc = _get_program()
    in_maps = _make_in_maps(voxel_logits, sem_prob_dense)
    res = run_bass_kernel_spmd(nc, in_maps, list(range(NCORES)))
    full = np.concatenate([res.results[c]["out"] for c in range(NCORES)],
                          axis=1)
    return full.reshape(Q, X, Y, Z).astype(np.float32)
